# revision 1
# baseline (speedup 1.0000x reference)
"""Trainium2 Bass kernel for nn_MultiAttention (3-branch causal attention).

Reference math (B=4, S=1024, D=64), per batch b:
  br0: s = x @ x^T                      ; causal softmax ; o = P @ x
  br1: s = (x Wq^T)(x Wk^T + bk)^T * sc ; causal softmax ; o = P @ (x Wv^T)
  br2: s[q,k] = sum_d tanh(x[q,d]+x[k,d]); causal softmax ; o = P @ x
  out = w0*o0 + w1*o1 + w2*o2,  w = attn_w/sum(attn_w)

Sharding: 8 cores = 4 batches x 2 key-roles. Core (b, r) handles ALL 1024
queries of batch b against the interleaved 128-key blocks {2c+r : c<4}
(512 keys, gathered contiguously by the host). Causality at block level is
handled with a uniform (role-independent) slot structure: q-tile i visits
n(i) = i//2+1 local key chunks; role-dependent validity is pushed into
data (additive masks). Each core emits unnormalized flash-softmax partials
(m, l, o~) per branch; the host merges the two key-roles exactly.

Branch-2 (the additive-tanh branch) runs as pure matmul via a sine
series: tanh(z) ~ sum_m b_m sin(m pi z / L) on |z| <= 9.9 (max err 6e-6,
L=12, M=28), so with u = x_q[d], v = x_k[d]:
   sum_d tanh(u+v) = sum_m b_m [ <sin(w_m u), cos(w_m v)>_d
                               + <cos(w_m u), sin(w_m v)>_d ].
Per m: one DVE 2-op tensor_scalar builds w = (x + shift_m)/P_m, a second
applies the fp32 magic-number round, GPSIMD/DVE subtracts to fold the
argument into [-1/2, 1/2] periods, one ACT Sin (scale 2pi, per-partition
quarter-period shifts put sin on rows 0-63 and cos on rows 64-127)
produces the [128, S] feature tile in fp32r, and one K=128 fp32r matmul
per q-tile accumulates scores into a per-tile PSUM bank across all m.
"""

import os
import sys

import numpy as np

try:
    import concourse.bass  # noqa: F401  (ambient install, e.g. under axon)
except ImportError:  # fall back to the in-container checkout
    for _p in ("/opt/trn_rl_repo",):
        if _p not in sys.path and os.path.isdir(_p):
            sys.path.insert(0, _p)

B, S, D = 4, 1024, 64
QT = 128                       # q-tile rows
NQT = S // QT                  # 8 q-tiles
NKC = 4                        # local key chunks per core
KL = NKC * 128                 # 512 local keys per core
NEG = -30000.0                 # mask value (exp-safe in fp32)
N_OF = [i // 2 + 1 for i in range(NQT)]          # chunks visited per q-tile
SLOT0 = np.concatenate([[0], np.cumsum(N_OF)])   # mask slot offsets
NSLOT = int(SLOT0[-1])                           # 20
FL = 12.0                      # sine-series half-period for tanh approx
FM = int(os.environ.get('FM_OVERRIDE', 28))  # number of sine frequencies
MAGIC = 12582912.0             # 1.5 * 2**23: fp32 round-to-nearest trick

def _fit_tanh_sine(L=FL, M=FM, Zm=9.9):
    """Weighted least-squares fit: tanh(z) ~ sum_m b_m sin(m pi z / L)."""
    z = np.linspace(0, Zm, 40001)
    m = np.arange(1, M + 1)
    A = np.sin(np.outer(z, m * np.pi / L))
    wgt = np.ones_like(z)
    e = np.zeros_like(z)
    for _ in range(14):
        b, *_ = np.linalg.lstsq(A * wgt[:, None], np.tanh(z) * wgt, rcond=None)
        e = A @ b - np.tanh(z)
        wgt = np.sqrt(wgt * (np.abs(e) / np.abs(e).max() + 0.03))
        wgt /= wgt.max()
    return b


_prog_cache = {}
last_results = None  # BassKernelResults of the most recent run (for test.py)


def _build_program():
    import concourse.bacc as bacc
    import concourse.bass as bass
    import concourse.mybir as mybir
    import concourse.tile as tile
    from contextlib import ExitStack

    f32 = mybir.dt.float32
    f32r = mybir.dt.float32r
    AF = mybir.ActivationFunctionType
    ALU = mybir.AluOpType
    AX = mybir.AxisListType
    ts = bass.ts

    nc = bacc.Bacc("TRN2", target_bir_lowering=False, debug=False, num_devices=8)

    # ---- DRAM I/O ----
    d_xqt = nc.dram_tensor("xqt", [D, S], f32r, kind="ExternalInput").ap()
    d_x2a = nc.dram_tensor("x2a", [64, S + KL], f32, kind="ExternalInput").ap()
    d_shifts = nc.dram_tensor("shifts", [128, FM], f32,
                              kind="ExternalInput").ap()
    d_xkt = nc.dram_tensor("xkt", [D, KL], f32r, kind="ExternalInput").ap()
    d_xk = nc.dram_tensor("xk", [128, NKC, D], f32r, kind="ExternalInput").ap()
    d_masks = nc.dram_tensor("masks", [128, NQT, 128], f32,
                             kind="ExternalInput").ap()
    d_wqt = nc.dram_tensor("wqt", [D, D], f32r, kind="ExternalInput").ap()
    d_wkt = nc.dram_tensor("wkt", [D, D], f32r, kind="ExternalInput").ap()
    d_wvt = nc.dram_tensor("wvt", [D, D], f32r, kind="ExternalInput").ap()
    d_bk = nc.dram_tensor("bk", [D, 1], f32, kind="ExternalInput").ap()
    d_ident = nc.dram_tensor("ident", [128, 128], f32, kind="ExternalInput").ap()

    d_ot = nc.dram_tensor("ot", [3, D, S], f32, kind="ExternalOutput").ap()
    d_dbg = (nc.dram_tensor("dbg", [NQT, 128, 512], f32,
                            kind="ExternalOutput").ap()
             if os.environ.get("DEBUG_ACC") else None)
    d_dbgf = (nc.dram_tensor("dbgf", [2, 128, S + KL + 512], f32,
                             kind="ExternalOutput").ap()
              if os.environ.get("DEBUG_FEAT") else None)
    d_ml = nc.dram_tensor("ml", [NQT, 128, 6], f32, kind="ExternalOutput").ap()

    with tile.TileContext(nc) as tc, ExitStack() as ctx:
        consts = ctx.enter_context(tc.tile_pool(name="consts", bufs=1))
        accp = ctx.enter_context(tc.tile_pool(name="accp", bufs=1))
        fwp = ctx.enter_context(tc.tile_pool(name="fwp", bufs=4))
        ffp = ctx.enter_context(tc.tile_pool(name="ffp", bufs=5))
        smp = ctx.enter_context(tc.tile_pool(name="smp", bufs=2))
        pp = ctx.enter_context(tc.tile_pool(name="pp", bufs=3))
        ptsp = ctx.enter_context(tc.tile_pool(name="ptsp", bufs=2))
        osp = ctx.enter_context(tc.tile_pool(name="osp", bufs=3))
        mlp = ctx.enter_context(tc.tile_pool(name="mlp", bufs=12))
        ps = ctx.enter_context(tc.tile_pool(name="ps", bufs=2, space="PSUM"))

        # ---- load constants ----
        def load(tag, shape, src, dt=f32):
            t = consts.tile(shape, dt, tag=tag)
            nc.sync.dma_start(t[:], src)
            return t

        x2a = consts.tile([128, S + KL], f32, tag="x2a")
        nc.sync.dma_start(x2a[0:64, :], d_x2a)
        nc.sync.dma_start(x2a[64:128, :], d_x2a)
        shifts = load("shifts", [128, FM], d_shifts)
        xqt = load("xqt", [D, S], d_xqt, f32r)
        xkt = load("xkt", [D, KL], d_xkt, f32r)
        xk = load("xk", [128, NKC, D], d_xk, f32r)
        masks = load("masks", [128, NQT, 128], d_masks)
        wqt = load("wqt", [D, D], d_wqt, f32r)
        wkt = load("wkt", [D, D], d_wkt, f32r)
        wvt = load("wvt", [D, D], d_wvt, f32r)
        bk = load("bk", [D, 1], d_bk)
        ident = load("ident", [128, 128], d_ident)

        # ---- projections: qt = (Wq' x^T), kt = (Wk xk^T + bk), v = xk Wv^T ----
        qt = consts.tile([D, S], f32r)
        for h in range(2):
            qp = ps.tile([D, 512], f32, tag="s3p0", bufs=1)
            nc.tensor.matmul(qp[:], wqt[:], xqt[:, ts(h, 512)],
                             start=True, stop=True)
            nc.scalar.copy(qt[:, ts(h, 512)], qp[:])
        kt = consts.tile([D, KL], f32r)
        kp = ps.tile([D, KL], f32, tag="s3p1", bufs=1)
        nc.tensor.matmul(kp[:], wkt[:], xkt[:], start=True, stop=True)
        nc.scalar.activation(kt[:], kp[:], AF.Identity, bias=bk[:, 0:1])
        vt = consts.tile([128, NKC, D], f32r)
        for c in range(NKC):
            vp = ps.tile([128, D], f32, tag="s3p2", bufs=1)
            nc.tensor.matmul(vp[:], xkt[:, ts(c, 128)], wvt[:],
                             start=True, stop=True)
            nc.scalar.copy(vt[:, c, :], vp[:])

        # ---- branch-2 scores via sine-series features ----
        # tanh(zq+zk) ~ sum_m b_m [sin(w_m zq) cos(w_m zk) + cos(w_m zq) sin(w_m zk)]
        # Per m: fold args into [-1/2, 1/2] periods with the fp32 round trick,
        # one ACT Sin produces [sin;cos] feature rows (per-partition shifts),
        # then one K=128 fp32r matmul per q-tile accumulates into PSUM.
        bcoef = _fit_tanh_sine()
        s3ps = []
        for i in range(NQT):
            s3pt = ps.tile([128, 512], f32, tag=f"s3p{i}", bufs=1,
                           name=f"s3p{i}")
            s3ps.append(s3pt)
        W = S + KL
        for mi in range(FM):
            mval = mi + 1
            pm = 2.0 * FL / mval
            wt = fwp.tile([128, W], f32, tag="wt")
            nc.vector.tensor_scalar(wt[:], x2a[:], shifts[:, mi:mi + 1],
                                    float(1.0 / pm), ALU.add, ALU.mult)
            rt = fwp.tile([128, W], f32, tag="rt")
            nc.vector.tensor_scalar(rt[:], wt[:], MAGIC, MAGIC,
                                    ALU.add, ALU.subtract)
            dt_ = fwp.tile([128, W], f32, tag="dt")
            eng = nc.vector if mi % 5 == 4 else nc.gpsimd
            eng.tensor_tensor(dt_[:], wt[:], rt[:], ALU.subtract)
            ft = ffp.tile([128, W], f32r, tag="ft")
            nc.scalar.activation(ft[:], dt_[:], AF.Sin,
                                 scale=float(2.0 * np.pi))
            # key-side features swapped: [b*cos_k ; b*sin_k] so the K=128
            # contraction yields sin(w(zq+zk)) = sin*cos + cos*sin
            fk = ffp.tile([128, KL], f32r, tag="fk")
            nc.vector.tensor_scalar_mul(fk[0:64, :], ft[64:128, S:S + KL],
                                        float(bcoef[mi]))
            nc.vector.tensor_scalar_mul(fk[64:128, :], ft[0:64, S:S + KL],
                                        float(bcoef[mi]))
            if d_dbgf is not None and mi in (0, 5):
                j = 0 if mi == 0 else 1
                nc.sync.dma_start(d_dbgf[j, :, :S + KL], ft[:].bitcast(f32))
                nc.sync.dma_start(d_dbgf[j, :, S + KL:], fk[:].bitcast(f32))
            for i in range(NQT):
                klp = 128 * max(N_OF[i], 2)
                nc.tensor.matmul(s3ps[i][:, :klp], ft[:, ts(i, 128)],
                                 fk[:, :klp], start=(mi == 0),
                                 stop=(mi == FM - 1), skip_group_check=True)


        # drain PSUM score accumulators to SBUF (frees all banks)
        accs = []
        for i in range(NQT):
            a = accp.tile([128, 512], f32, tag=f"acc{i}")
            nc.scalar.copy(a[:, :128 * N_OF[i]], s3ps[i][:, :128 * N_OF[i]])
            accs.append(a)

        # ---- per-q-tile branches: mask (final chunk only), softmax, PV ----
        # br2 first: consuming s3p_i frees its PSUM bank for br0/br1 scores
        for i in range(NQT):
            n = N_OF[i]
            kl = 128 * n
            ovt3 = ps.tile([D, 3, 128], f32,
                           tag=f"s3p{(3 * i + 2) % 8}", bufs=1)
            mlt = mlp.tile([128, 6], f32, tag="mlt")
            for br in (2, 0, 1):
                if br == 2:
                    sp = accs[i]
                else:
                    sp = ps.tile([128, 512], f32,
                                 tag=f"s3p{(3 * i) % 8}", bufs=1)
                    lhs = xqt[:, ts(i, 128)] if br == 0 else qt[:, ts(i, 128)]
                    rhs = xkt if br == 0 else kt
                    nc.tensor.matmul(sp[:, :kl], lhs, rhs[:, :kl],
                                     start=True, stop=True)
                # causal mask applies only to the final local chunk
                nc.vector.tensor_tensor(sp[:, kl - 128:kl], sp[:, kl - 128:kl],
                                        masks[:, i, :], ALU.add)
                mt = mlp.tile([128, 1], f32, tag="mt")
                nc.vector.reduce_max(mt[:], sp[:, :kl], axis=AX.X)
                nmt = mlp.tile([128, 1], f32, tag="nmt")
                nc.vector.tensor_scalar_mul(nmt[:], mt[:], -1.0)
                pt = pp.tile([128, 512], f32, tag="pt")
                lt = mlp.tile([128, 1], f32, tag="lt")
                nc.scalar.activation(pt[:, :kl], sp[:, :kl], AF.Exp,
                                     bias=nmt[:, 0:1], accum_out=lt[:, 0:1])
                if br == 2 and d_dbg is not None:
                    nc.sync.dma_start(d_dbg[i], sp[:])
                # P^T chunks via PE transpose into one PSUM bank, one copy
                ptp = ps.tile([128, 512], f32,
                              tag=f"s3p{(3 * i + 1) % 8}", bufs=1)
                for c in range(n):
                    nc.tensor.transpose(ptp[:, ts(c, 128)], pt[:, ts(c, 128)],
                                        ident[:])
                pts = ptsp.tile([128, 512], f32r, tag="pts")
                nc.scalar.copy(pts[:, :kl], ptp[:, :kl])
                vsrc = vt if br == 1 else xk
                for c in range(n):
                    nc.tensor.matmul(ovt3[:, br, :], vsrc[:, c, :],
                                     pts[:, ts(c, 128)],
                                     start=(c == 0), stop=(c == n - 1))
                nc.vector.tensor_copy(mlt[:, 2 * br:2 * br + 1], mt[:])
                nc.vector.tensor_copy(mlt[:, 2 * br + 1:2 * br + 2], lt[:])

            ost = osp.tile([D, 3, 128], f32, tag="ost")
            nc.scalar.copy(ost[:], ovt3[:])
            for br in range(3):
                nc.sync.dma_start(d_ot[br, :, ts(i, 128)], ost[:, br, :])
            nc.sync.dma_start(d_ml[i], mlt[:])

    nc.compile()
    return nc


def _get_prog():
    if "nc" not in _prog_cache:
        _prog_cache["nc"] = _build_program()
    return _prog_cache["nc"]


def _host_inputs(x, Wq, Wk, bk, Wv, attn_scale):
    """Build the 8 per-core input maps."""
    x = np.ascontiguousarray(np.asarray(x, dtype=np.float32))
    sc = float(np.asarray(attn_scale).reshape(-1)[0]) / np.sqrt(D)
    wqt = np.ascontiguousarray(np.asarray(Wq, np.float32).T * sc)
    wkt = np.ascontiguousarray(np.asarray(Wk, np.float32).T)
    wvt = np.ascontiguousarray(np.asarray(Wv, np.float32).T)
    bkc = np.ascontiguousarray(np.asarray(bk, np.float32).reshape(D, 1))
    ident = np.eye(128, dtype=np.float32)

    # per-frequency fold shifts: c_m (multiple of the period, keeps the
    # mod-input positive) plus quarter-period on the cos half (rows 64-127)
    shifts = np.zeros((128, FM), np.float32)
    for mi in range(FM):
        mval = mi + 1
        pm = 2.0 * FL / mval
        cm = pm * np.ceil(6.0 / pm)
        shifts[:64, mi] = cm
        shifts[64:, mi] = cm + FL / (2.0 * mval)

    qi = np.arange(128)[:, None]
    ki = np.arange(128)[None, :]

    in_maps = []
    for b in range(B):
        xb = x[b]                          # [S, D]
        xbt = np.ascontiguousarray(xb.T)   # [D, S]
        for role in range(2):
            gblocks = [2 * c + role for c in range(NKC)]
            xk_g = np.concatenate([xb[128 * g:128 * g + 128] for g in gblocks])
            xkt_g = np.ascontiguousarray(xk_g.T)          # [D, KL]
            xk_c = np.ascontiguousarray(
                xk_g.reshape(NKC, 128, D).transpose(1, 0, 2))  # [128, NKC, D]
            x2a = np.empty((64, S + KL), np.float32)
            x2a[:, :S] = xbt
            x2a[:, S:] = xkt_g

            # mask for the final local chunk of each q-tile (all earlier
            # chunks are fully valid): g = 2*(n-1)+role vs tile i
            masks = np.zeros((128, NQT, 128), np.float32)
            for i in range(NQT):
                g = 2 * (N_OF[i] - 1) + role
                if g == i:
                    masks[:, i, :] = np.where(ki <= qi, 0.0, NEG)
                elif g > i:
                    masks[:, i, :] = NEG
            in_maps.append({
                "xqt": xbt, "x2a": x2a, "shifts": shifts,
                "xkt": xkt_g, "xk": xk_c,
                "masks": masks, "wqt": wqt, "wkt": wkt, "wvt": wvt,
                "bk": bkc, "ident": ident,
            })
    return in_maps


def _merge(results, attn_w):
    """Exact flash-softmax merge of the two key-role partials per batch."""
    w = np.asarray(attn_w, np.float64)
    w = w / w.sum()
    out = np.zeros((B, S, D), np.float64)
    for b in range(B):
        ra, rb = results[2 * b], results[2 * b + 1]
        for br in range(3):
            ma = ra["ml"][:, :, 2 * br].reshape(S).astype(np.float64)
            mb = rb["ml"][:, :, 2 * br].reshape(S).astype(np.float64)
            la = ra["ml"][:, :, 2 * br + 1].reshape(S).astype(np.float64)
            lb = rb["ml"][:, :, 2 * br + 1].reshape(S).astype(np.float64)
            oa = ra["ot"][br].T.astype(np.float64)   # [S, D]
            ob = rb["ot"][br].T.astype(np.float64)
            m = np.maximum(ma, mb)
            pa = np.exp(ma - m)
            pb = np.exp(mb - m)
            # fully-masked partials have garbage l/o but p == 0 exactly
            num = (np.where(pa[:, None] > 0, pa[:, None] * oa, 0.0)
                   + np.where(pb[:, None] > 0, pb[:, None] * ob, 0.0))
            den = np.where(pa > 0, pa * la, 0.0) + np.where(pb > 0, pb * lb, 0.0)
            out[b] += w[br] * (num / den[:, None])
    return out.astype(np.float32)


def kernel(x, Wq, Wk, bk, Wv, attn_w, attn_scale):
    global last_results
    from concourse.bass_utils import run_bass_kernel_spmd

    nc = _get_prog()
    in_maps = _host_inputs(x, Wq, Wk, bk, Wv, attn_scale)
    trace = os.environ.get("BASS_TRACE_KERNEL", "0") == "1"
    res = run_bass_kernel_spmd(nc, in_maps, core_ids=list(range(8)),
                               trace=trace)
    last_results = res
    return _merge(res.results, attn_w)


if __name__ == "__main__":
    rng = np.random.default_rng(0)
    xs = rng.standard_normal((B, S, D), dtype=np.float32)
    out = kernel(xs,
                 rng.standard_normal((D, D), dtype=np.float32) / 8,
                 rng.standard_normal((D, D), dtype=np.float32) / 8,
                 rng.standard_normal((D,), dtype=np.float32) / 8,
                 rng.standard_normal((D, D), dtype=np.float32) / 8,
                 np.ones(3, np.float32), np.ones(1, np.float32))
    print(out.shape, out.dtype)



# revision 11
# speedup vs baseline: 3.0588x; 3.0588x over previous
"""Trainium2 Bass kernel for nn_MultiAttention (3-branch causal attention).

Reference math (B=4, S=1024, D=64), per batch b:
  br0: s = x @ x^T                      ; causal softmax ; o = P @ x
  br1: s = (x Wq^T)(x Wk^T + bk)^T * sc ; causal softmax ; o = P @ (x Wv^T)
  br2: s[q,k] = sum_d tanh(x[q,d]+x[k,d]); causal softmax ; o = P @ x
  out = w0*o0 + w1*o1 + w2*o2,  w = attn_w/sum(attn_w)

Sharding: 8 cores = 4 batches x 2 key-roles. Core (b, r) handles all 1024
queries of batch b against the interleaved 128-key blocks {2c+r : c<4}.
The host permutes the query column order per core so key blocks always sit
at even block positions; the SPMD program is role-independent and the host
merge unpermutes.

Design (all scores computed TRANSPOSED, s^T[k, q], q-tiles of 256):
- No on-device row max: softmax uses host-computed per-query upper bounds
  C[q] (Cauchy-Schwarz / prefix-max bounds over each query's full diagonal
  block), subtracted inside the score matmul itself via an extra
  contraction row (ones x -C) or a rank-1 accumulate. Both key-roles share
  C so the host merge is a plain sum: out = (o_a + o_b) / (l_a + l_b).
- l comes free from PV: V is extended with a ones column, so PV's output
  row 64 is the softmax denominator.
- Causality: only each q-tile's diagonal chunk needs masking; applied as a
  0/1 multiply on P^T (post-exp, on Pool) - C bounds cover the whole
  diagonal block so unmasked entries cannot overflow.
- Branch-2 runs as pure matmul via a free-frequency sine fit:
  tanh(z) ~ sum_m b_m sin(om_m z) (max err 1.2e-3 on |z|<=9.6, M=8), and
  sin(a+b) = sin(a+pi/4)sin(b+pi/4) - sin(a+3pi/4)sin(b+3pi/4), so one
  feature tile per m serves both q and k sides (keys are a gathered subset
  of query columns; the +/-b_m key scaling is one per-partition-scalar op).
  Range reduction per m: one DVE tensor_scalar (x/P + phase), one magic-
  number round, and the subtract folded into the PE (+I/-I accumulate).
- ACT table thrash avoided: all Sin ops complete before any Exp op.
"""

import os
import sys

import numpy as np

try:
    import concourse.bass  # noqa: F401  (ambient install, e.g. under axon)
except ImportError:  # fall back to the in-container checkout
    for _p in ("/opt/trn_rl_repo",):
        if _p not in sys.path and os.path.isdir(_p):
            sys.path.insert(0, _p)

B, S, D = 4, 1024, 64
QT = 256                       # q-tile width
NT = S // QT                   # 4 q-tiles
NKC = 4                        # local key chunks per core
KL = NKC * 128                 # 512 local keys per core
FM = 8                         # sine-series terms
MAGIC = 12582912.0             # 1.5 * 2**23: fp32 round-to-nearest trick
SSCALE = float(2.0 * np.pi * (1.0 - 5e-7))

# free-frequency LSQ fit of tanh on [0, 9.6] (max err 1.21e-3)
OMEGAS = [0.2734280786, 0.8243559956, 1.3856134054, 1.9598657311,
          2.5472323275, 3.1465182453, 3.7546312203, 4.3568228756]
BCOEF = [1.23654055, 0.3289342548, 0.1304462844, 0.0535883686,
         0.0217261607, 0.0086277304, 0.0033462421, 0.001215308]

# br2 chain groups: one open PSUM accumulation chain per bank.
# (bank, first_tile, n_tiles, chunk): out width = 256*n_tiles
B2CHAINS = [(0, 0, 2, 0),   # tiles 0-1, chunk 0
            (1, 2, 2, 0),   # tiles 2-3, chunk 0
            (2, 2, 2, 1),   # tiles 2-3, chunk 1
            (3, 2, 2, 2),   # tiles 2-3, chunk 2
            (4, 1, 1, 1),   # tile 1, chunk 1 (diag)
            (5, 3, 1, 3)]   # tile 3, chunk 3 (diag)
# (tile, chunk) -> (bank, col offset) for PV / mask lookups
B2REG = {}
for _bk, _t0, _nt, _c in B2CHAINS:
    for _j in range(_nt):
        B2REG[(_t0 + _j, _c)] = (_bk, 256 * _j)
B2DIAG = {i: B2REG[(i, i)] for i in range(NT)}

# blobr (f32r): xq2 | +I | -I ; blob (f32): phi | bvec | tri01
OFF_PI = S
OFF_NI = S + 128
BLOBRW = S + 256
OFF_PHI = 0
OFF_BV = 1
OFF_TRI = 1 + FM
BLOBW = 1 + FM + 256

_prog_cache = {}
last_results = None  # BassKernelResults of the most recent run (for test.py)


def _build_program():
    import concourse.bacc as bacc
    import concourse.bass as bass
    import concourse.mybir as mybir
    import concourse.tile as tile
    from contextlib import ExitStack

    f32 = mybir.dt.float32
    f32r = mybir.dt.float32r
    AF = mybir.ActivationFunctionType
    ALU = mybir.AluOpType
    ts = bass.ts

    nc = bacc.Bacc("TRN2", target_bir_lowering=False, debug=False,
                   num_devices=8)

    d_blobr = nc.dram_tensor("blobr", [128, BLOBRW], f32r,
                             kind="ExternalInput").ap()
    d_blob = nc.dram_tensor("blob", [128, BLOBW], f32,
                            kind="ExternalInput").ap()
    d_xqe = nc.dram_tensor("xqe", [65, S], f32r, kind="ExternalInput").ap()
    d_cr = nc.dram_tensor("cr", [33, S + 128], f32r,
                          kind="ExternalInput").ap()
    d_xke = nc.dram_tensor("xke", [65, KL], f32r, kind="ExternalInput").ap()
    d_xkx = nc.dram_tensor("xkx", [128, NKC, 65], f32r,
                           kind="ExternalInput").ap()
    d_w = nc.dram_tensor("wb", [64, 193], f32r, kind="ExternalInput").ap()

    dbg = os.environ.get("DEBUG_K", "0") == "1"
    d_dbg = (nc.dram_tensor("dbg", [4, 128, S], f32,
                            kind="ExternalOutput").ap() if dbg else None)
    d_dbs = (nc.dram_tensor("dbs", [6, 128, 512], f32,
                            kind="ExternalOutput").ap() if dbg else None)
    d_o01 = nc.dram_tensor("o01", [NT, 65, 512], f32,
                           kind="ExternalOutput").ap()
    d_o2 = nc.dram_tensor("o2", [2, 65, 512], f32,
                          kind="ExternalOutput").ap()

    with tile.TileContext(nc) as tc, ExitStack() as ctx:
        consts = ctx.enter_context(tc.tile_pool(name="consts", bufs=1))
        a2p = ctx.enter_context(tc.tile_pool(name="a2p", bufs=2))
        rmp = ctx.enter_context(tc.tile_pool(name="rmp", bufs=2))
        ftp = ctx.enter_context(tc.tile_pool(name="ftp", bufs=3))
        fkp = ctx.enter_context(tc.tile_pool(name="fkp", bufs=3))
        ptsp = ctx.enter_context(tc.tile_pool(name="ptsp", bufs=1))
        osp = ctx.enter_context(tc.tile_pool(name="osp", bufs=1))
        ps = ctx.enter_context(tc.tile_pool(name="ps", bufs=1, space="PSUM"))

        blobr = consts.tile([128, BLOBRW], f32r, tag="blobr")
        nc.sync.dma_start(blobr[:], d_blobr)
        crows = consts.tile([33, S + 128], f32r, tag="crows")
        nc.sync.dma_start(crows[:], d_cr)
        blob = consts.tile([128, BLOBW], f32, tag="blob")
        nc.sync.dma_start(blob[:], d_blob)
        wb = consts.tile([64, 193], f32r, tag="wb")
        nc.sync.dma_start(wb[:], d_w)
        xke = consts.tile([65, KL], f32r, tag="xke")
        nc.sync.dma_start(xke[:], d_xke)
        xkx = consts.tile([128, NKC, 65], f32r, tag="xkx")
        nc.sync.dma_start(xkx[:], d_xkx)
        xqe = consts.tile([65, S], f32r, tag="xqe")
        nc.sync.dma_start(xqe[:], d_xqe)

        xq2 = blobr[:, 0:S]
        phi = blob[:, OFF_PHI:OFF_PHI + 1]
        bvec = blob[:, OFF_BV:OFF_BV + FM]
        tri01 = blob[:, OFF_TRI:OFF_TRI + 256].bitcast(f32r)
        posI = blobr[:, OFF_PI:OFF_PI + 128]
        negI = blobr[:, OFF_NI:OFF_NI + 128]
        negC1 = crows[0:1, 0:S]            # -C1 row (base partition 0)
        negC2 = crows[32:33, 0:S]          # -C2 row (base partition 32)
        ones1 = crows[0:1, S:S + 128]      # ones, base 0 (pairs with -C1)
        ones2 = crows[32:33, S:S + 128]    # ones, base 32 (pairs with -C2)

        qt = consts.tile([64, S], f32r, tag="qt")
        kt = consts.tile([64, KL], f32r, tag="kt")
        vte = consts.tile([128, NKC, 65], f32r, tag="vte")

        # br2 score banks 0-5 (persist through the m-loop; one open
        # accumulation chain per bank)
        psb = [ps.tile([128, 512], f32, tag=f"psb{i}", bufs=1, name=f"psb{i}")
               for i in range(6)]

        # ---- feature m-loop (phase A) ----
        # a = x/P + phase ; r = round(a) [magic] ; d = a - r on the PE via
        # +I/-I accumulate (banks 5-7 rotate) ; f = sin(2*pi*d) ;
        # fk = (+/-b_m) * f[key cols]
        rot = [0]

        def dslot(shape):
            t = ps.tile(shape, f32, tag=f"psd{rot[0] % 2}", bufs=1,
                        name=f"psd{rot[0] % 2}")
            rot[0] += 1
            return t

        def emit_proj():
            # qt = Wq' x^T (scaled), kt = Wk' x^T + bk, vte = x Wv^T | 1
            for h in range(2):
                qp = dslot([64, 512])
                nc.tensor.matmul(qp[:], wb[:, 0:64], xq2[0:64, ts(h, 512)],
                                 start=True, stop=True)
                nc.vector.tensor_copy(qt[:, ts(h, 512)], qp[:])
            kp = dslot([64, KL])
            nc.tensor.matmul(kp[:], wb[:, 64:128], xke[0:64, :],
                             start=True, stop=True)
            nc.vector.tensor_scalar(kt[:], kp[:],
                                    wb[:, 192:193].bitcast(f32), None,
                                    ALU.add)
            vp = dslot([128, 256])
            for c in range(NKC):
                nc.tensor.matmul(vp[:, ts(c, 64)], xke[0:64, ts(c, 128)],
                                 wb[:, 128:192], start=True, stop=True)
                nc.vector.tensor_copy(vte[:, c, 0:64], vp[:, ts(c, 64)])
            nc.vector.tensor_copy(vte[:, :, 64:65], xkx[:, :, 64:65])

        for mi in range(FM):
            pm = 2.0 * np.pi / OMEGAS[mi]
            a2 = a2p.tile([128, S], f32r, tag="a2")
            nc.vector.tensor_scalar(a2[:], xq2[:].bitcast(f32),
                                    float(1.0 / pm), phi[:, 0:1],
                                    ALU.mult, ALU.add)
            rm = rmp.tile([128, S], f32r, tag="rm")
            nc.vector.tensor_scalar(rm[:], a2[:].bitcast(f32),
                                    MAGIC, MAGIC, ALU.add, ALU.subtract)
            ft = ftp.tile([128, NT, 2, 128], f32r, tag="ft")
            for sl in range(2):
                dt_ = dslot([128, 2, 2, 128])
                nc.tensor.matmul(dt_[:], posI, a2[:, ts(sl, 512)],
                                 start=True, stop=False,
                                 skip_group_check=True)
                nc.tensor.matmul(dt_[:], negI, rm[:, ts(sl, 512)],
                                 start=False, stop=True,
                                 skip_group_check=True)
                nc.scalar.activation(ft[:, 2 * sl:2 * sl + 2, :, :], dt_[:],
                                     AF.Sin, scale=SSCALE)
            if d_dbg is not None and mi == 2:
                nc.sync.dma_start(d_dbg[0], a2[:].bitcast(f32))
                nc.sync.dma_start(d_dbg[1], rm[:].bitcast(f32))
                nc.sync.dma_start(d_dbg[2], ft[:].bitcast(f32))
            fk = fkp.tile([128, NKC, 128], f32r, tag="fk")
            nc.vector.tensor_scalar(fk[:], ft[:, :, 0, :].bitcast(f32),
                                    bvec[:, mi:mi + 1], None, ALU.mult)
            if d_dbg is not None and mi == 2:
                nc.sync.dma_start(d_dbg[3, :, 0:512],
                                  fk[:].bitcast(f32))
            for bk_, t0, nt_, c in B2CHAINS:
                nc.tensor.matmul(psb[bk_][:, 0:256 * nt_], fk[:, c, :],
                                 ft[:, t0:t0 + nt_, :, :], start=(mi == 0),
                                 stop=(mi == FM - 1), skip_group_check=True)
            if mi == 1:
                emit_proj()
            if mi == 2:
                for bk_, t0, nt_, c in B2CHAINS:
                    nc.tensor.matmul(psb[bk_][:, 0:256 * nt_], ones2,
                                     negC2[:, 256 * t0:256 * (t0 + nt_)],
                                     start=False, stop=False,
                                     skip_group_check=True)

        # ---- phase B: exp, mask, PV, drain ----
        # br2 exps first (one per bank), then per-tile br0/br1 units.
        if d_dbs is not None:
            for bk_ in range(6):
                dcp = ptsp.tile([128, 512], f32, tag="dcp", bufs=2, name="dcp")
                nc.vector.tensor_copy(dcp[:], psb[bk_][:])
                nc.sync.dma_start(d_dbs[bk_], dcp[:])
        pts2 = [None] * 6
        for bk_, t0, nt_, c in sorted(B2CHAINS, key=lambda g: -g[0]):
            p2 = ptsp.tile([128, 512], f32r, tag=f"pts2{bk_}", bufs=1,
                           name=f"pts2{bk_}")
            nc.scalar.activation(p2[:, 0:256 * nt_], psb[bk_][:, 0:256 * nt_],
                                 AF.Exp)
            pts2[bk_] = p2
        for i, (bk_, off) in B2DIAG.items():
            nc.vector.tensor_tensor(pts2[bk_][:, off:off + 256],
                                    pts2[bk_][:, off:off + 256],
                                    tri01, ALU.mult)

        if d_dbg is not None:
            # post-mask view of g0's P tile (after all br2 PVs are emitted
            # this slot in emission order is late enough)
            pass

        def pts_of(i, c):
            bk_, off = B2REG[(i, c)]
            return pts2[bk_][:, off:off + 256]

        # br0/br1 per tile: scores into psd0/psd1 banks, exp, mask,
        # PV into psd2
        for i in range(NT):
            n = i + 1
            pv = ps.tile([65, 512], f32, tag="psb4", bufs=1, name="pv")
            for br in range(2):
                sb5 = ps.tile([128, 512], f32, tag="psd0", bufs=1, name="sb5")
                sb6 = (ps.tile([128, 512], f32, tag="psd1", bufs=1,
                               name="sb6") if n > 2 else None)
                regions = [(sb5, 0), (sb5, 256), (sb6, 0), (sb6, 256)][:n]
                for c, (sb, off) in enumerate(regions):
                    if br == 0:
                        nc.tensor.matmul(sb[:, off:off + 256],
                                         xke[:, ts(c, 128)],
                                         xqe[:, ts(i, 256)],
                                         start=True, stop=True,
                                         skip_group_check=True)
                    else:
                        nc.tensor.matmul(sb[:, off:off + 256],
                                         kt[:, ts(c, 128)],
                                         qt[:, ts(i, 256)],
                                         start=True, stop=False,
                                         skip_group_check=True)
                        nc.tensor.matmul(sb[:, off:off + 256], ones1,
                                         negC1[:, ts(i, 256)], start=False,
                                         stop=True, skip_group_check=True)
                p01 = ptsp.tile([128, 512], f32r, tag="p01a", bufs=2,
                                name="p01")
                nc.scalar.activation(p01[:, 0:256 * min(n, 2)],
                                     sb5[:, 0:256 * min(n, 2)], AF.Exp)
                p01b = None
                if n > 2:
                    p01b = ptsp.tile([128, 512], f32r, tag="p01b", bufs=2,
                                     name="p01b")
                    nc.scalar.activation(p01b[:, 0:256 * (n - 2)],
                                         sb6[:, 0:256 * (n - 2)], AF.Exp)
                # mask the diagonal chunk (c = i)
                dptile = p01 if i < 2 else p01b
                doff = (i % 2) * 256
                nc.vector.tensor_tensor(dptile[:, doff:doff + 256],
                                        dptile[:, doff:doff + 256],
                                        tri01, ALU.mult)
                vsrc = xkx if br == 0 else vte
                for c in range(n):
                    pcs = p01 if c < 2 else p01b
                    poff = (c % 2) * 256
                    nc.tensor.matmul(pv[:, ts(br, 256)], vsrc[:, c, :],
                                     pcs[:, poff:poff + 256],
                                     start=(c == 0), stop=(c == n - 1),
                                     skip_group_check=True)
            ot = osp.tile([65, 512], f32, tag="ot", bufs=3, name="ot")
            nc.vector.tensor_copy(ot[:], pv[:])
            nc.sync.dma_start(d_o01[i], ot[:])

            # interleave br2 PV pairs after tiles 0 and 1
            if i == 0:
                pv2a = ps.tile([65, 512], f32, tag="psb0", bufs=1,
                               name="pv2a")
                nc.tensor.matmul(pv2a[:, 0:256], xkx[:, 0, :], pts_of(0, 0),
                                 start=True, stop=True, skip_group_check=True)
                for c in range(2):
                    nc.tensor.matmul(pv2a[:, 256:512], xkx[:, c, :],
                                     pts_of(1, c), start=(c == 0),
                                     stop=(c == 1), skip_group_check=True)
                ot2a = osp.tile([65, 512], f32, tag="ot2a", bufs=1,
                                name="ot2a")
                nc.vector.tensor_copy(ot2a[:], pv2a[:])
                nc.sync.dma_start(d_o2[0], ot2a[:])
            if i == 1:
                pv2b = ps.tile([65, 512], f32, tag="psb1", bufs=1,
                               name="pv2b")
                for c in range(3):
                    nc.tensor.matmul(pv2b[:, 0:256], xkx[:, c, :],
                                     pts_of(2, c), start=(c == 0),
                                     stop=(c == 2), skip_group_check=True)
                for c in range(4):
                    nc.tensor.matmul(pv2b[:, 256:512], xkx[:, c, :],
                                     pts_of(3, c), start=(c == 0),
                                     stop=(c == 3), skip_group_check=True)
                ot2b = osp.tile([65, 512], f32, tag="ot2b", bufs=1,
                                name="ot2b")
                nc.vector.tensor_copy(ot2b[:], pv2b[:])
                nc.sync.dma_start(d_o2[1], ot2b[:])
                if d_dbg is not None:
                    nc.sync.dma_start(d_dbg[0, :, 0:512],
                                      pts2[0][:].bitcast(f32))

    nc.compile()
    return nc


def _get_prog():
    if "nc" not in _prog_cache:
        _prog_cache["nc"] = _build_program()
    return _prog_cache["nc"]


def _perm_idx(role):
    perm = list(range(8)) if role == 0 else [1, 0, 3, 2, 5, 4, 7, 6]
    return np.concatenate([np.arange(128 * g, 128 * (g + 1)) for g in perm])


def _host_inputs(x, Wq, Wk, bk, Wv, attn_scale):
    """Build the 8 per-core input maps."""
    x = np.ascontiguousarray(np.asarray(x, dtype=np.float32))
    sc = float(np.asarray(attn_scale).reshape(-1)[0]) / np.sqrt(D)
    Wq = np.asarray(Wq, np.float32)
    Wk = np.asarray(Wk, np.float32)
    Wv = np.asarray(Wv, np.float32)
    bkc = np.asarray(bk, np.float32).reshape(D)

    wb = np.zeros((64, 193), np.float32)
    wb[:, 0:64] = Wq.T * sc
    wb[:, 64:128] = Wk.T
    wb[:, 128:192] = Wv.T
    wb[:, 192] = bkc

    # mask[partition=k, col=q] = 1 iff key k <= query q (within block)
    kk = np.arange(128)[:, None]
    qq = np.arange(128)[None, :]
    tril128 = (kk <= qq).astype(np.float32)

    # C bounds must cover every key the device exponentiates unmasked:
    # tile i processes key blocks up to 2i+1 (role 1), so cover through the
    # end of the odd block of each query's block pair.
    blk_end = np.minimum(128 * (((np.arange(S) // 128) | 1) + 1) - 1, S - 1)

    in_maps = []
    for b in range(B):
        xb = x[b]                          # [S, D]

        # per-query exp-offset bounds (cover the full diagonal block)
        nx = np.linalg.norm(xb, axis=1)
        C0 = nx * np.maximum.accumulate(nx)[blk_end] + 0.1
        qm = xb @ Wq.T * sc
        km = xb @ Wk.T + bkc
        C1 = (np.linalg.norm(qm, axis=1)
              * np.maximum.accumulate(np.linalg.norm(km, axis=1))[blk_end]
              + 0.1)
        Mblk = np.maximum.accumulate(xb, axis=0)[blk_end]
        C2 = np.tanh(xb + Mblk).sum(axis=1) + 0.5

        for role in range(2):
            pidx = _perm_idx(role)
            xpt = np.ascontiguousarray(xb[pidx].T)   # [D, S] permuted
            gblocks = [2 * c + role for c in range(NKC)]
            xk_g = np.concatenate([xb[128 * g:128 * g + 128] for g in gblocks])

            blobr = np.zeros((128, BLOBRW), np.float32)
            blobr[0:64, 0:S] = xpt
            blobr[64:128, 0:S] = xpt
            blobr[:, OFF_PI:OFF_PI + 128] = np.eye(128, dtype=np.float32)
            blobr[:, OFF_NI:OFF_NI + 128] = -np.eye(128, dtype=np.float32)
            blob = np.zeros((128, BLOBW), np.float32)
            blob[0:64, OFF_PHI] = 0.125
            blob[64:128, OFF_PHI] = 0.375
            for mi in range(FM):
                blob[0:64, OFF_BV + mi] = BCOEF[mi]
                blob[64:128, OFF_BV + mi] = -BCOEF[mi]
            blob[:, OFF_TRI:OFF_TRI + 128] = tril128
            blob[:, OFF_TRI + 128:OFF_TRI + 256] = 1.0 if role == 0 else 0.0

            xqe = np.zeros((65, S), np.float32)
            xqe[0:64] = xpt
            xqe[64] = -C0[pidx]
            cr = np.zeros((33, S + 128), np.float32)
            cr[0, 0:S] = -C1[pidx]
            cr[32, 0:S] = -C2[pidx]
            cr[0, S:] = 1.0
            cr[32, S:] = 1.0

            xke = np.ones((65, KL), np.float32)
            xke[0:64] = xk_g.T

            xkx = np.ones((128, NKC, 65), np.float32)
            xkx[:, :, 0:64] = xk_g.reshape(NKC, 128, D).transpose(1, 0, 2)

            in_maps.append({"blobr": blobr, "blob": blob, "xqe": xqe,
                            "xke": xke, "xkx": xkx, "wb": wb, "cr": cr})
    return in_maps


def _merge(results, attn_w):
    """Merge the two key-role partials per batch (shared C offsets)."""
    w = np.asarray(attn_w, np.float64)
    w = w / w.sum()
    out = np.zeros((B, S, D), np.float64)
    for b in range(B):
        for br in range(3):
            o = np.zeros((S, 64), np.float64)
            l = np.zeros(S, np.float64)
            for role in range(2):
                r = results[2 * b + role]
                pidx = _perm_idx(role)
                op = np.zeros((S, 64), np.float64)
                lp = np.zeros(S, np.float64)
                for i in range(NT):
                    if br < 2:
                        seg = r["o01"][i][:, 256 * br:256 * br + 256]
                    else:
                        seg = r["o2"][i // 2][:, 256 * (i % 2):
                                              256 * (i % 2) + 256]
                    op[QT * i:QT * (i + 1)] = seg[0:64].T
                    lp[QT * i:QT * (i + 1)] = seg[64]
                o[pidx] += op
                l[pidx] += lp
            out[b] += w[br] * (o / l[:, None])
    return out.astype(np.float32)


def kernel(x, Wq, Wk, bk, Wv, attn_w, attn_scale):
    global last_results
    from concourse.bass_utils import run_bass_kernel_spmd

    nc = _get_prog()
    in_maps = _host_inputs(x, Wq, Wk, bk, Wv, attn_scale)
    trace = os.environ.get("BASS_TRACE_KERNEL", "0") == "1"
    res = run_bass_kernel_spmd(nc, in_maps, core_ids=list(range(8)),
                               trace=trace)
    last_results = res
    return _merge(res.results, attn_w)


if __name__ == "__main__":
    rng = np.random.default_rng(0)
    xs = rng.standard_normal((B, S, D), dtype=np.float32)
    out = kernel(xs,
                 rng.standard_normal((D, D), dtype=np.float32) / 8,
                 rng.standard_normal((D, D), dtype=np.float32) / 8,
                 rng.standard_normal((D,), dtype=np.float32) / 8,
                 rng.standard_normal((D, D), dtype=np.float32) / 8,
                 np.ones(3, np.float32), np.ones(1, np.float32))
    print(out.shape, out.dtype)


# revision 13
# speedup vs baseline: 3.4052x; 1.1132x over previous
"""Trainium2 Bass kernel for nn_MultiAttention (3-branch causal attention).

Reference math (B=4, S=1024, D=64), per batch b:
  br0: s = x @ x^T                      ; causal softmax ; o = P @ x
  br1: s = (x Wq^T)(x Wk^T + bk)^T * sc ; causal softmax ; o = P @ (x Wv^T)
  br2: s[q,k] = sum_d tanh(x[q,d]+x[k,d]); causal softmax ; o = P @ x
  out = w0*o0 + w1*o1 + w2*o2,  w = attn_w/sum(attn_w)

Sharding: 8 cores = 4 batches x 2 key-roles. Core (b, r) handles all 1024
queries of batch b against the interleaved 128-key blocks {2c+r : c<4}.
The host permutes the query column order per core so key blocks always sit
at even block positions; the SPMD program is role-independent and the host
merge unpermutes.

Design (all scores computed TRANSPOSED, s^T[k, q], q-tiles of 256):
- No on-device row max: softmax uses host-computed per-query upper bounds
  C[q] (Cauchy-Schwarz / prefix-max bounds over each query's full diagonal
  block pair), subtracted inside the score matmul itself via an extra
  contraction row (ones x -C) or an in-chain rank-1 accumulate. Both
  key-roles share C so the host merge is a plain sum:
  out = (o_a + o_b) / (l_a + l_b).
- l comes free from PV: V is extended with a ones column, so PV's output
  row 64 is the softmax denominator.
- Causality: only each q-tile's diagonal chunk needs masking; applied as a
  0/1 multiply on P^T (post-exp) - C bounds cover the whole diagonal block
  pair so unmasked entries cannot overflow.
- Branch-2 runs as pure matmul via a free-frequency sine fit:
  tanh(z) ~ sum_m b_m sin(om_m z) (max err 1.2e-3 on |z|<=9.6, M=8), and
  sin(a+b) = sin(a+pi/4)sin(b+pi/4) - sin(a+3pi/4)sin(b+3pi/4), so one
  feature tile per m serves both q and k sides (keys are a gathered subset
  of query columns; the +/-b_m key scaling is one per-partition-scalar op).
  Range reduction per m: one DVE tensor_scalar (x/P + phase), one magic-
  number round, and the subtract split between the PE (+I/-I accumulate,
  slab 0) and Pool (tensor_tensor, slab 1).
- PSUM accumulation is chain-based (one OPEN chain per bank): br2 scores
  use 6 chains in banks 0-5 (adjacent tiles sharing a chunk pair into one
  512-wide chain); banks 6-7 (psd0/1) rotate for the d slabs, projections,
  and phase-B score tiles.
- Phase A is software-pipelined in emission order (fk lags one m, feature
  matmuls lag two) so no engine queue head-waits on its own iteration.
- ACT table thrash avoided: all Sin ops complete before any Exp op.
"""

import os
import sys

import numpy as np

try:
    import concourse.bass  # noqa: F401  (ambient install, e.g. under axon)
except ImportError:  # fall back to the in-container checkout
    for _p in ("/opt/trn_rl_repo",):
        if _p not in sys.path and os.path.isdir(_p):
            sys.path.insert(0, _p)

B, S, D = 4, 1024, 64
QT = 256                       # q-tile width
NT = S // QT                   # 4 q-tiles
NKC = 4                        # local key chunks per core
KL = NKC * 128                 # 512 local keys per core
FM = 8                         # sine-series terms
MAGIC = 12582912.0             # 1.5 * 2**23: fp32 round-to-nearest trick
SSCALE = float(2.0 * np.pi * (1.0 - 5e-7))

# free-frequency LSQ fit of tanh on [0, 9.6] (max err 1.21e-3)
OMEGAS = [0.2734280786, 0.8243559956, 1.3856134054, 1.9598657311,
          2.5472323275, 3.1465182453, 3.7546312203, 4.3568228756]
BCOEF = [1.23654055, 0.3289342548, 0.1304462844, 0.0535883686,
         0.0217261607, 0.0086277304, 0.0033462421, 0.001215308]

# br2 chain groups: one open PSUM accumulation chain per bank.
# (bank, first_tile, n_tiles, chunk): out width = 256*n_tiles
B2CHAINS = [(0, 0, 2, 0),   # tiles 0-1, chunk 0
            (1, 2, 2, 0),   # tiles 2-3, chunk 0
            (2, 2, 2, 1),   # tiles 2-3, chunk 1
            (3, 2, 2, 2),   # tiles 2-3, chunk 2
            (4, 1, 1, 1),   # tile 1, chunk 1 (diag)
            (5, 3, 1, 3)]   # tile 3, chunk 3 (diag)
B2REG = {}
for _bk, _t0, _nt, _c in B2CHAINS:
    for _j in range(_nt):
        B2REG[(_t0 + _j, _c)] = (_bk, 256 * _j)
B2DIAG = {i: B2REG[(i, i)] for i in range(NT)}

# blobr (f32r): xq2 | +I | -I ; blob (f32): phi | bvec | tri01
OFF_PI = S
OFF_NI = S + 128
BLOBRW = S + 256
OFF_PHI = 0
OFF_BV = 1
OFF_TRI = 1 + FM
BLOBW = 1 + FM + 256
CRW = S + 128 + KL             # crows: -C1/-C2 | ones128 | ones512

_prog_cache = {}
last_results = None  # BassKernelResults of the most recent run (for test.py)


def _build_program():
    import concourse.bacc as bacc
    import concourse.bass as bass
    import concourse.mybir as mybir
    import concourse.tile as tile
    from contextlib import ExitStack

    f32 = mybir.dt.float32
    f32r = mybir.dt.float32r
    AF = mybir.ActivationFunctionType
    ALU = mybir.AluOpType
    ts = bass.ts

    nc = bacc.Bacc("TRN2", target_bir_lowering=False, debug=False,
                   num_devices=8)

    d_blob = nc.dram_tensor("blob", [128, BLOBW], f32,
                            kind="ExternalInput").ap()
    d_pmi = nc.dram_tensor("pmi", [128, 256], f32r,
                           kind="ExternalInput").ap()
    d_xq2 = nc.dram_tensor("xq2", [64, S], f32r, kind="ExternalInput").ap()
    d_xqe = nc.dram_tensor("xqe", [65, S], f32r, kind="ExternalInput").ap()
    d_cr = nc.dram_tensor("cr", [33, CRW], f32r, kind="ExternalInput").ap()
    d_xke = nc.dram_tensor("xke", [65, KL], f32r, kind="ExternalInput").ap()
    d_xkx = nc.dram_tensor("xkx", [128, NKC, 65], f32r,
                           kind="ExternalInput").ap()
    d_w = nc.dram_tensor("wb", [64, 193], f32r, kind="ExternalInput").ap()

    d_o01 = nc.dram_tensor("o01", [NT, 65, 512], f32,
                           kind="ExternalOutput").ap()
    d_o2 = nc.dram_tensor("o2", [2, 65, 512], f32,
                          kind="ExternalOutput").ap()

    with tile.TileContext(nc) as tc, ExitStack() as ctx:
        consts = ctx.enter_context(tc.tile_pool(name="consts", bufs=1))
        a2p = ctx.enter_context(tc.tile_pool(name="a2p", bufs=2))
        rmp = ctx.enter_context(tc.tile_pool(name="rmp", bufs=2))
        d1p = ctx.enter_context(tc.tile_pool(name="d1p", bufs=2))
        ftp = ctx.enter_context(tc.tile_pool(name="ftp", bufs=5))
        fkp = ctx.enter_context(tc.tile_pool(name="fkp", bufs=5))
        ptsp = ctx.enter_context(tc.tile_pool(name="ptsp", bufs=1))
        osp = ctx.enter_context(tc.tile_pool(name="osp", bufs=1))
        ps = ctx.enter_context(tc.tile_pool(name="ps", bufs=1, space="PSUM"))

        # DMAs in priority order (phi/bvec/tri + +/-I + xq2 gate the m-loop)
        blob = consts.tile([128, BLOBW], f32, tag="blob")
        nc.sync.dma_start(blob[:], d_blob)
        pmi = consts.tile([128, 256], f32r, tag="pmi")
        nc.sync.dma_start(pmi[:], d_pmi)
        xq2 = consts.tile([128, S], f32r, tag="xq2")
        nc.sync.dma_start(xq2[0:64, :], d_xq2)
        nc.sync.dma_start(xq2[64:128, :], d_xq2)
        wb = consts.tile([64, 193], f32r, tag="wb")
        nc.sync.dma_start(wb[:], d_w)
        xke = consts.tile([65, KL], f32r, tag="xke")
        nc.sync.dma_start(xke[:], d_xke)
        crows = consts.tile([33, CRW], f32r, tag="crows")
        nc.sync.dma_start(crows[:], d_cr)
        xkx = consts.tile([128, NKC, 65], f32r, tag="xkx")
        nc.sync.dma_start(xkx[:], d_xkx)
        xqe = consts.tile([65, S], f32r, tag="xqe")
        nc.sync.dma_start(xqe[:], d_xqe)

        qt = consts.tile([65, S], f32r, tag="qt")
        nc.sync.dma_start(qt[64:65, :], d_cr[0:1, 0:S])       # -C1 row
        kt = consts.tile([65, KL], f32r, tag="kt")
        nc.sync.dma_start(kt[64:65, :], d_cr[0:1, S + 128:])  # ones row
        vte = consts.tile([128, NKC, 65], f32r, tag="vte")

        phi = blob[:, OFF_PHI:OFF_PHI + 1]
        bvec = blob[:, OFF_BV:OFF_BV + FM]
        tri01 = blob[:, OFF_TRI:OFF_TRI + 256].bitcast(f32r)
        posI = pmi[:, 0:128]
        negI = pmi[:, 128:256]
        negC2 = crows[32:33, 0:S]          # -C2 row (base partition 32)
        ones2 = crows[32:33, S:S + 128]    # ones, base 32 (pairs with -C2)

        # br2 score banks 0-5; psd0/1 rotate for d slabs / proj / phase B
        psb = [ps.tile([128, 512], f32, tag=f"psb{i}", bufs=1, name=f"psb{i}")
               for i in range(6)]

        def psd(k, shape=[128, 2, 2, 128]):
            return ps.tile(shape, f32, tag=f"psd{k % 2}", bufs=1,
                           name=f"psd{k % 2}")

        # ---- projections (fills early PE idle while DMAs land) ----
        # qt = Wq' x^T (scaled), kt = Wk' x^T + bk, vte = x Wv^T | 1
        for h in range(2):
            qp = psd(h, [64, 512])
            nc.tensor.matmul(qp[:], wb[:, 0:64], xq2[0:64, ts(h, 512)],
                             start=True, stop=True)
            nc.vector.tensor_copy(qt[0:64, ts(h, 512)], qp[:])
        kp = psd(0, [64, KL])
        nc.tensor.matmul(kp[:], wb[:, 64:128], xke[0:64, :],
                         start=True, stop=True)
        nc.vector.tensor_scalar(kt[0:64, :], kp[:],
                                wb[:, 192:193].bitcast(f32), None, ALU.add)
        vp = psd(1, [128, 256])
        for c in range(NKC):
            nc.tensor.matmul(vp[:, ts(c, 64)], xke[0:64, ts(c, 128)],
                             wb[:, 128:192], start=True, stop=True)
            nc.vector.tensor_copy(vte[:, c, 0:64], vp[:, ts(c, 64)])
        nc.vector.tensor_copy(vte[:, :, 64:65], xkx[:, :, 64:65])

        # ---- feature m-loop (phase A), software-pipelined emission ----
        # a = x/P + phase ; r = round(a) [magic] ; d = a - r (slab0 on PE
        # via +I/-I, slab1 on Pool) ; f = sin(2*pi*d) ; fk = (+/-b_m)*f[key]
        a2s, rms, d0s, d1s, fts, fks = {}, {}, {}, {}, {}, {}

        def emit_head(m):
            pm = 2.0 * np.pi / OMEGAS[m]
            a2 = a2p.tile([128, S], f32r, tag="a2")
            nc.vector.tensor_scalar(a2[:], xq2[:].bitcast(f32),
                                    float(1.0 / pm), phi[:, 0:1],
                                    ALU.mult, ALU.add)
            rm = rmp.tile([128, S], f32r, tag="rm")
            nc.vector.tensor_scalar(rm[:], a2[:].bitcast(f32),
                                    MAGIC, MAGIC, ALU.add, ALU.subtract)
            a2s[m], rms[m] = a2, rm
            dt0 = psd(m)
            nc.tensor.matmul(dt0[:], posI, a2[:, 0:512],
                             start=True, stop=False, skip_group_check=True)
            nc.tensor.matmul(dt0[:], negI, rm[:, 0:512],
                             start=False, stop=True, skip_group_check=True)
            d0s[m] = dt0
            dt1 = d1p.tile([128, 2, 2, 128], f32, tag="d1")
            nc.gpsimd.tensor_tensor(dt1[:].rearrange("p a b c -> p (a b c)"),
                                    a2[:, 512:1024].bitcast(f32),
                                    rm[:, 512:1024].bitcast(f32),
                                    ALU.subtract)
            d1s[m] = dt1

        def emit_sin(m):
            ft = ftp.tile([128, NT, 2, 128], f32r, tag="ft")
            nc.scalar.activation(ft[:, 0:2, :, :], d0s[m][:], AF.Sin,
                                 scale=SSCALE)
            nc.scalar.activation(ft[:, 2:4, :, :], d1s[m][:], AF.Sin,
                                 scale=SSCALE)
            fts[m] = ft

        def emit_fk(m):
            fk = fkp.tile([128, NKC, 128], f32r, tag="fk")
            nc.vector.tensor_scalar(fk[:], fts[m][:, :, 0, :].bitcast(f32),
                                    bvec[:, m:m + 1], None, ALU.mult)
            fks[m] = fk

        def emit_feat(m):
            for bk_, t0, nt_, c in B2CHAINS:
                nc.tensor.matmul(psb[bk_][:, 0:256 * nt_], fks[m][:, c, :],
                                 fts[m][:, t0:t0 + nt_, :, :],
                                 start=(m == 0), stop=(m == FM - 1),
                                 skip_group_check=True)
            if m == 1:
                for bk_, t0, nt_, c in B2CHAINS:
                    nc.tensor.matmul(psb[bk_][:, 0:256 * nt_], ones2,
                                     negC2[:, 256 * t0:256 * (t0 + nt_)],
                                     start=False, stop=False,
                                     skip_group_check=True)

        for m in range(FM):
            emit_head(m)
            if m >= 1:
                emit_sin(m - 1)
            if m >= 2:
                emit_fk(m - 2)
            if m >= 3:
                emit_feat(m - 3)
        emit_sin(FM - 1)
        for m in (FM - 2, FM - 1):
            emit_fk(m)
        for m in (FM - 3, FM - 2, FM - 1):
            emit_feat(m)

        # ---- phase B: exp, mask, PV, drain ----
        pts2 = [None] * 6
        for bk_, t0, nt_, c in B2CHAINS:
            p2 = ptsp.tile([128, 512], f32r, tag=f"pts2{bk_}", bufs=1,
                           name=f"pts2{bk_}")
            nc.scalar.activation(p2[:, 0:256 * nt_], psb[bk_][:, 0:256 * nt_],
                                 AF.Exp)
            pts2[bk_] = p2
        for i, (bk_, off) in B2DIAG.items():
            nc.gpsimd.tensor_tensor(pts2[bk_][:, off:off + 256],
                                    pts2[bk_][:, off:off + 256],
                                    tri01, ALU.mult)

        def pts_of(i, c):
            bk_, off = B2REG[(i, c)]
            return pts2[bk_][:, off:off + 256]

        # br2 PVs (banks psb0/psb1 reused after their exps)
        pv2a = ps.tile([65, 512], f32, tag="psb0", bufs=1, name="pv2a")
        nc.tensor.matmul(pv2a[:, 0:256], xkx[:, 0, :], pts_of(0, 0),
                         start=True, stop=True, skip_group_check=True)
        for c in range(2):
            nc.tensor.matmul(pv2a[:, 256:512], xkx[:, c, :], pts_of(1, c),
                             start=(c == 0), stop=(c == 1),
                             skip_group_check=True)
        ot2a = osp.tile([65, 512], f32, tag="ot2a", bufs=1, name="ot2a")
        nc.vector.tensor_copy(ot2a[:], pv2a[:])
        nc.sync.dma_start(d_o2[0], ot2a[:])
        pv2b = ps.tile([65, 512], f32, tag="psb1", bufs=1, name="pv2b")
        for c in range(3):
            nc.tensor.matmul(pv2b[:, 0:256], xkx[:, c, :], pts_of(2, c),
                             start=(c == 0), stop=(c == 2),
                             skip_group_check=True)
        for c in range(4):
            nc.tensor.matmul(pv2b[:, 256:512], xkx[:, c, :], pts_of(3, c),
                             start=(c == 0), stop=(c == 3),
                             skip_group_check=True)
        ot2b = osp.tile([65, 512], f32, tag="ot2b", bufs=1, name="ot2b")
        nc.vector.tensor_copy(ot2b[:], pv2b[:])
        nc.sync.dma_start(d_o2[1], ot2b[:])

        # br0/br1 per tile (big tiles first): per-branch 1-bank score tiles
        # (psd0/psd1, chunk slots rotate), per-chunk exp, mask, PV into
        # psb2's bank
        for i in (3, 2, 1, 0):
            n = i + 1
            pv = ps.tile([65, 512], f32, tag="psb2", bufs=1, name="pv")
            sbs = [ps.tile([128, 2, 256], f32, tag="psd0", bufs=1,
                           name="sb0"),
                   ps.tile([128, 2, 256], f32, tag="psd1", bufs=1,
                           name="sb1")]
            p01s = [ptsp.tile([128, NKC, 256], f32r, tag=f"p01{br}",
                              bufs=2, name=f"p01{br}") for br in range(2)]
            for c in range(n):
                for br in range(2):
                    lhs, rhs = (xke, xqe) if br == 0 else (kt, qt)
                    nc.tensor.matmul(sbs[br][:, c % 2, :],
                                     lhs[:, ts(c, 128)],
                                     rhs[0:65, ts(i, 256)],
                                     start=True, stop=True,
                                     skip_group_check=True)
                for br in range(2):
                    nc.scalar.activation(p01s[br][:, c, :],
                                         sbs[br][:, c % 2, :], AF.Exp)
            for br in range(2):
                nc.gpsimd.tensor_tensor(p01s[br][:, i, :], p01s[br][:, i, :],
                                        tri01, ALU.mult)
            for br in range(2):
                vsrc = xkx if br == 0 else vte
                for c in range(n):
                    nc.tensor.matmul(pv[:, ts(br, 256)], vsrc[:, c, :],
                                     p01s[br][:, c, :],
                                     start=(c == 0), stop=(c == n - 1),
                                     skip_group_check=True)
            ot = osp.tile([65, 512], f32, tag="ot", bufs=3, name="ot")
            nc.vector.tensor_copy(ot[:], pv[:])
            nc.sync.dma_start(d_o01[i], ot[:])

    nc.compile()
    return nc


def _get_prog():
    if "nc" not in _prog_cache:
        _prog_cache["nc"] = _build_program()
    return _prog_cache["nc"]


def _perm_idx(role):
    perm = list(range(8)) if role == 0 else [1, 0, 3, 2, 5, 4, 7, 6]
    return np.concatenate([np.arange(128 * g, 128 * (g + 1)) for g in perm])


def _host_inputs(x, Wq, Wk, bk, Wv, attn_scale):
    """Build the 8 per-core input maps."""
    x = np.ascontiguousarray(np.asarray(x, dtype=np.float32))
    sc = float(np.asarray(attn_scale).reshape(-1)[0]) / np.sqrt(D)
    Wq = np.asarray(Wq, np.float32)
    Wk = np.asarray(Wk, np.float32)
    Wv = np.asarray(Wv, np.float32)
    bkc = np.asarray(bk, np.float32).reshape(D)

    wb = np.zeros((64, 193), np.float32)
    wb[:, 0:64] = Wq.T * sc
    wb[:, 64:128] = Wk.T
    wb[:, 128:192] = Wv.T
    wb[:, 192] = bkc

    # mask[partition=k, col=q] = 1 iff key k <= query q (within block)
    kk = np.arange(128)[:, None]
    qq = np.arange(128)[None, :]
    tril128 = (kk <= qq).astype(np.float32)

    # C bounds must cover every key the device exponentiates unmasked:
    # tile i processes key blocks up to 2i+1 (role 1), so cover through the
    # end of the odd block of each query's block pair.
    blk_end = np.minimum(128 * (((np.arange(S) // 128) | 1) + 1) - 1, S - 1)

    pmi = np.zeros((128, 256), np.float32)
    pmi[:, 0:128] = np.eye(128, dtype=np.float32)
    pmi[:, 128:256] = -np.eye(128, dtype=np.float32)

    in_maps = []
    for b in range(B):
        xb = x[b]                          # [S, D]

        nx = np.linalg.norm(xb, axis=1)
        C0 = nx * np.maximum.accumulate(nx)[blk_end] + 0.1
        qm = xb @ Wq.T * sc
        km = xb @ Wk.T + bkc
        C1 = (np.linalg.norm(qm, axis=1)
              * np.maximum.accumulate(np.linalg.norm(km, axis=1))[blk_end]
              + 0.1)
        Mblk = np.maximum.accumulate(xb, axis=0)[blk_end]
        C2 = np.tanh(xb + Mblk).sum(axis=1) + 0.5

        for role in range(2):
            pidx = _perm_idx(role)
            xpt = np.ascontiguousarray(xb[pidx].T)   # [D, S] permuted
            gblocks = [2 * c + role for c in range(NKC)]
            xk_g = np.concatenate([xb[128 * g:128 * g + 128] for g in gblocks])

            blob = np.zeros((128, BLOBW), np.float32)
            blob[0:64, OFF_PHI] = 0.125
            blob[64:128, OFF_PHI] = 0.375
            for mi in range(FM):
                blob[0:64, OFF_BV + mi] = BCOEF[mi]
                blob[64:128, OFF_BV + mi] = -BCOEF[mi]
            blob[:, OFF_TRI:OFF_TRI + 128] = tril128
            blob[:, OFF_TRI + 128:OFF_TRI + 256] = 1.0 if role == 0 else 0.0

            xqe = np.zeros((65, S), np.float32)
            xqe[0:64] = xpt
            xqe[64] = -C0[pidx]
            cr = np.zeros((33, CRW), np.float32)
            cr[0, 0:S] = -C1[pidx]
            cr[32, 0:S] = -C2[pidx]
            cr[0, S:] = 1.0
            cr[32, S:] = 1.0

            xke = np.ones((65, KL), np.float32)
            xke[0:64] = xk_g.T

            xkx = np.ones((128, NKC, 65), np.float32)
            xkx[:, :, 0:64] = xk_g.reshape(NKC, 128, D).transpose(1, 0, 2)

            in_maps.append({"blob": blob, "pmi": pmi, "xq2": xpt,
                            "xqe": xqe, "xke": xke, "xkx": xkx,
                            "wb": wb, "cr": cr})
    return in_maps


def _merge(results, attn_w):
    """Merge the two key-role partials per batch (shared C offsets)."""
    w = np.asarray(attn_w, np.float64)
    w = w / w.sum()
    out = np.zeros((B, S, D), np.float64)
    for b in range(B):
        for br in range(3):
            o = np.zeros((S, 64), np.float64)
            l = np.zeros(S, np.float64)
            for role in range(2):
                r = results[2 * b + role]
                pidx = _perm_idx(role)
                op = np.zeros((S, 64), np.float64)
                lp = np.zeros(S, np.float64)
                for i in range(NT):
                    if br < 2:
                        seg = r["o01"][i][:, 256 * br:256 * br + 256]
                    else:
                        seg = r["o2"][i // 2][:, 256 * (i % 2):
                                              256 * (i % 2) + 256]
                    op[QT * i:QT * (i + 1)] = seg[0:64].T
                    lp[QT * i:QT * (i + 1)] = seg[64]
                o[pidx] += op
                l[pidx] += lp
            out[b] += w[br] * (o / l[:, None])
    return out.astype(np.float32)


def kernel(x, Wq, Wk, bk, Wv, attn_w, attn_scale):
    global last_results
    from concourse.bass_utils import run_bass_kernel_spmd

    nc = _get_prog()
    in_maps = _host_inputs(x, Wq, Wk, bk, Wv, attn_scale)
    trace = os.environ.get("BASS_TRACE_KERNEL", "0") == "1"
    res = run_bass_kernel_spmd(nc, in_maps, core_ids=list(range(8)),
                               trace=trace)
    last_results = res
    return _merge(res.results, attn_w)


if __name__ == "__main__":
    rng = np.random.default_rng(0)
    xs = rng.standard_normal((B, S, D), dtype=np.float32)
    out = kernel(xs,
                 rng.standard_normal((D, D), dtype=np.float32) / 8,
                 rng.standard_normal((D, D), dtype=np.float32) / 8,
                 rng.standard_normal((D,), dtype=np.float32) / 8,
                 rng.standard_normal((D, D), dtype=np.float32) / 8,
                 np.ones(3, np.float32), np.ones(1, np.float32))
    print(out.shape, out.dtype)


# revision 14
# speedup vs baseline: 3.7091x; 1.0892x over previous
"""Trainium2 Bass kernel for nn_MultiAttention (3-branch causal attention).

Reference math (B=4, S=1024, D=64), per batch b:
  br0: s = x @ x^T                      ; causal softmax ; o = P @ x
  br1: s = (x Wq^T)(x Wk^T + bk)^T * sc ; causal softmax ; o = P @ (x Wv^T)
  br2: s[q,k] = sum_d tanh(x[q,d]+x[k,d]); causal softmax ; o = P @ x
  out = w0*o0 + w1*o1 + w2*o2,  w = attn_w/sum(attn_w)

Sharding: 8 cores = 4 batches x 2 key-roles. Core (b, r) handles all 1024
queries of batch b against the interleaved 128-key blocks {2c+r : c<4}.
The host permutes the query column order per core so key blocks always sit
at even block positions; the SPMD program is role-independent and the host
merge unpermutes.

Design (all scores computed TRANSPOSED, s^T[k, q], q-tiles of 256):
- No on-device row max: softmax uses host-computed per-query upper bounds
  C[q] (Cauchy-Schwarz / prefix-max bounds over each query's full diagonal
  block pair), subtracted inside the score matmul itself via an extra
  contraction row (ones x -C) or an in-chain rank-1 accumulate. Both
  key-roles share C so the host merge is a plain sum:
  out = (o_a + o_b) / (l_a + l_b).
- l comes free from PV: V is extended with a ones column, so PV's output
  row 64 is the softmax denominator.
- Causality: only each q-tile's diagonal chunk needs masking; applied as a
  0/1 multiply on P^T (post-exp) - C bounds cover the whole diagonal block
  pair so unmasked entries cannot overflow.
- Branch-2 runs as pure matmul via a free-frequency sine fit:
  tanh(z) ~ sum_m b_m sin(om_m z) (max err 1.2e-3 on |z|<=9.6, M=8), and
  sin(a+b) = sin(a+pi/4)sin(b+pi/4) - sin(a+3pi/4)sin(b+3pi/4), so one
  feature tile per m serves both q and k sides (keys are a gathered subset
  of query columns; the +/-b_m key scaling is one per-partition-scalar op).
  Range reduction per m: one DVE tensor_scalar (x/P + phase), one magic-
  number round, and the subtract split between the PE (+I/-I accumulate,
  slab 0) and Pool (tensor_tensor, slab 1).
- PSUM accumulation is chain-based (one OPEN chain per bank): br2 scores
  use 6 chains in banks 0-5 (adjacent tiles sharing a chunk pair into one
  512-wide chain); banks 6-7 (psd0/1) rotate for the d slabs, projections,
  and phase-B score tiles.
- Phase A is software-pipelined in emission order (fk lags one m, feature
  matmuls lag two) so no engine queue head-waits on its own iteration.
- ACT table thrash avoided: all Sin ops complete before any Exp op.
"""

import os
import sys

import numpy as np

try:
    import concourse.bass  # noqa: F401  (ambient install, e.g. under axon)
except ImportError:  # fall back to the in-container checkout
    for _p in ("/opt/trn_rl_repo",):
        if _p not in sys.path and os.path.isdir(_p):
            sys.path.insert(0, _p)

B, S, D = 4, 1024, 64
QT = 256                       # q-tile width
NT = S // QT                   # 4 q-tiles
NKC = 4                        # local key chunks per core
KL = NKC * 128                 # 512 local keys per core
FM = 8                         # sine-series terms
MAGIC = 12582912.0             # 1.5 * 2**23: fp32 round-to-nearest trick
SSCALE = float(2.0 * np.pi * (1.0 - 5e-7))

# free-frequency LSQ fit of tanh on [0, 9.6] (max err 1.21e-3)
OMEGAS = [0.2734280786, 0.8243559956, 1.3856134054, 1.9598657311,
          2.5472323275, 3.1465182453, 3.7546312203, 4.3568228756]
BCOEF = [1.23654055, 0.3289342548, 0.1304462844, 0.0535883686,
         0.0217261607, 0.0086277304, 0.0033462421, 0.001215308]

# br2 chain groups: one open PSUM accumulation chain per bank.
# (bank, first_tile, n_tiles, chunk): out width = 256*n_tiles
B2CHAINS = [(0, 0, 2, 0),   # tiles 0-1, chunk 0
            (1, 2, 2, 0),   # tiles 2-3, chunk 0
            (2, 2, 2, 1),   # tiles 2-3, chunk 1
            (3, 2, 2, 2),   # tiles 2-3, chunk 2
            (4, 1, 1, 1),   # tile 1, chunk 1 (diag)
            (5, 3, 1, 3)]   # tile 3, chunk 3 (diag)
B2REG = {}
for _bk, _t0, _nt, _c in B2CHAINS:
    for _j in range(_nt):
        B2REG[(_t0 + _j, _c)] = (_bk, 256 * _j)
B2DIAG = {i: B2REG[(i, i)] for i in range(NT)}

# blobr (f32r): xq2 | +I | -I ; blob (f32): phi | bvec | tri01
OFF_PI = S
OFF_NI = S + 128
BLOBRW = S + 256
OFF_PHI = 0
OFF_BV = 1
OFF_TRI = 1 + FM
BLOBW = 1 + FM + 256
CRW = S + 128 + KL             # crows: -C1/-C2 | ones128 | ones512

_prog_cache = {}
last_results = None  # BassKernelResults of the most recent run (for test.py)


def _build_program():
    import concourse.bacc as bacc
    import concourse.bass as bass
    import concourse.mybir as mybir
    import concourse.tile as tile
    from contextlib import ExitStack

    f32 = mybir.dt.float32
    f32r = mybir.dt.float32r
    AF = mybir.ActivationFunctionType
    ALU = mybir.AluOpType
    ts = bass.ts

    nc = bacc.Bacc("TRN2", target_bir_lowering=False, debug=False,
                   num_devices=8)

    d_blob = nc.dram_tensor("blob", [128, BLOBW], f32,
                            kind="ExternalInput").ap()
    d_pmi = nc.dram_tensor("pmi", [128, 256], f32r,
                           kind="ExternalInput").ap()
    d_xq2 = nc.dram_tensor("xq2", [64, S], f32r, kind="ExternalInput").ap()
    d_xqe = nc.dram_tensor("xqe", [65, S], f32r, kind="ExternalInput").ap()
    d_cr = nc.dram_tensor("cr", [33, CRW], f32r, kind="ExternalInput").ap()
    d_xke = nc.dram_tensor("xke", [65, KL], f32r, kind="ExternalInput").ap()
    d_xkx = nc.dram_tensor("xkx", [128, NKC, 65], f32r,
                           kind="ExternalInput").ap()
    d_w = nc.dram_tensor("wb", [64, 193], f32r, kind="ExternalInput").ap()

    d_o01 = nc.dram_tensor("o01", [NT, 65, 512], f32,
                           kind="ExternalOutput").ap()
    d_o2 = nc.dram_tensor("o2", [2, 65, 512], f32,
                          kind="ExternalOutput").ap()

    with tile.TileContext(nc) as tc, ExitStack() as ctx:
        consts = ctx.enter_context(tc.tile_pool(name="consts", bufs=1))
        a2p = ctx.enter_context(tc.tile_pool(name="a2p", bufs=2))
        rmp = ctx.enter_context(tc.tile_pool(name="rmp", bufs=2))
        d1p = ctx.enter_context(tc.tile_pool(name="d1p", bufs=2))
        ftp = ctx.enter_context(tc.tile_pool(name="ftp", bufs=5))
        fkp = ctx.enter_context(tc.tile_pool(name="fkp", bufs=5))
        ptsp = ctx.enter_context(tc.tile_pool(name="ptsp", bufs=1))
        osp = ctx.enter_context(tc.tile_pool(name="osp", bufs=1))
        ps = ctx.enter_context(tc.tile_pool(name="ps", bufs=1, space="PSUM"))

        # DMAs in priority order (phi/bvec/tri + +/-I + xq2 gate the m-loop)
        blob = consts.tile([128, BLOBW], f32, tag="blob")
        nc.sync.dma_start(blob[:], d_blob)
        pmi = consts.tile([128, 256], f32r, tag="pmi")
        nc.sync.dma_start(pmi[:], d_pmi)
        wb = consts.tile([64, 193], f32r, tag="wb")
        nc.sync.dma_start(wb[:], d_w)
        xke = consts.tile([65, KL], f32r, tag="xke")
        nc.sync.dma_start(xke[:], d_xke)
        xq2 = consts.tile([128, S], f32r, tag="xq2")
        nc.sync.dma_start(xq2[0:64, :], d_xq2)
        nc.sync.dma_start(xq2[64:128, :], d_xq2)
        crows = consts.tile([33, CRW], f32r, tag="crows")
        nc.sync.dma_start(crows[:], d_cr)
        xkx = consts.tile([128, NKC, 65], f32r, tag="xkx")
        nc.sync.dma_start(xkx[:], d_xkx)
        xqe = consts.tile([65, S], f32r, tag="xqe")
        nc.sync.dma_start(xqe[:], d_xqe)

        qt = consts.tile([65, S], f32r, tag="qt")
        nc.sync.dma_start(qt[64:65, :], d_cr[0:1, 0:S])       # -C1 row
        kt = consts.tile([65, KL], f32r, tag="kt")
        nc.sync.dma_start(kt[64:65, :], d_cr[0:1, S + 128:])  # ones row
        vte = consts.tile([128, NKC, 65], f32r, tag="vte")

        phi = blob[:, OFF_PHI:OFF_PHI + 1]
        bvec = blob[:, OFF_BV:OFF_BV + FM]
        tri01 = blob[:, OFF_TRI:OFF_TRI + 256].bitcast(f32r)
        posI = pmi[:, 0:128]
        negI = pmi[:, 128:256]
        negC2 = crows[32:33, 0:S]          # -C2 row (base partition 32)
        ones2 = crows[32:33, S:S + 128]    # ones, base 32 (pairs with -C2)

        # br2 score banks 0-5; psd0/1 rotate for d slabs / proj / phase B
        psb = [ps.tile([128, 512], f32, tag=f"psb{i}", bufs=1, name=f"psb{i}")
               for i in range(6)]

        def psd(k, shape=[128, 2, 2, 128]):
            return ps.tile(shape, f32, tag=f"psd{k % 2}", bufs=1,
                           name=f"psd{k % 2}")

        # ---- projections (fills early PE idle while DMAs land) ----
        # qt = Wq' x^T (scaled), kt = Wk' x^T + bk, vte = x Wv^T | 1
        for h in range(2):
            qp = psd(h, [64, 512])
            nc.tensor.matmul(qp[:], wb[:, 0:64], xq2[0:64, ts(h, 512)],
                             start=True, stop=True)
            nc.scalar.activation(qt[0:64, ts(h, 512)], qp[:], AF.Identity)
        kp = psd(0, [64, KL])
        nc.tensor.matmul(kp[:], wb[:, 64:128], xke[0:64, :],
                         start=True, stop=True)
        nc.scalar.activation(kt[0:64, :], kp[:], AF.Identity,
                             bias=wb[:, 192:193].bitcast(f32))
        vp = psd(1, [128, 256])
        for c in range(NKC):
            nc.tensor.matmul(vp[:, ts(c, 64)], xke[0:64, ts(c, 128)],
                             wb[:, 128:192], start=True, stop=True)
            nc.scalar.activation(vte[:, c, 0:64], vp[:, ts(c, 64)],
                                 AF.Identity)
        nc.vector.tensor_copy(vte[:, :, 64:65], xkx[:, :, 64:65])

        # ---- feature m-loop (phase A), software-pipelined emission ----
        # a = x/P + phase ; r = round(a) [magic] ; d = a - r (slab0 on PE
        # via +I/-I, slab1 on Pool) ; f = sin(2*pi*d) ; fk = (+/-b_m)*f[key]
        a2s, rms, d0s, d1s, fts, fks = {}, {}, {}, {}, {}, {}

        def emit_head(m):
            pm = 2.0 * np.pi / OMEGAS[m]
            a2 = a2p.tile([128, S], f32r, tag="a2")
            nc.vector.tensor_scalar(a2[:], xq2[:].bitcast(f32),
                                    float(1.0 / pm), phi[:, 0:1],
                                    ALU.mult, ALU.add)
            rm = rmp.tile([128, S], f32r, tag="rm")
            nc.vector.tensor_scalar(rm[:], a2[:].bitcast(f32),
                                    MAGIC, MAGIC, ALU.add, ALU.subtract)
            a2s[m], rms[m] = a2, rm
            dt0 = psd(m)
            nc.tensor.matmul(dt0[:], posI, a2[:, 0:512],
                             start=True, stop=False, skip_group_check=True)
            nc.tensor.matmul(dt0[:], negI, rm[:, 0:512],
                             start=False, stop=True, skip_group_check=True)
            d0s[m] = dt0
            dt1 = d1p.tile([128, 2, 2, 128], f32, tag="d1")
            nc.gpsimd.tensor_tensor(dt1[:].rearrange("p a b c -> p (a b c)"),
                                    a2[:, 512:1024].bitcast(f32),
                                    rm[:, 512:1024].bitcast(f32),
                                    ALU.subtract)
            d1s[m] = dt1

        def emit_sin(m):
            ft = ftp.tile([128, NT, 2, 128], f32r, tag="ft")
            nc.scalar.activation(ft[:, 0:2, :, :], d0s[m][:], AF.Sin,
                                 scale=SSCALE)
            nc.scalar.activation(ft[:, 2:4, :, :], d1s[m][:], AF.Sin,
                                 scale=SSCALE)
            fts[m] = ft

        def emit_fk(m):
            fk = fkp.tile([128, NKC, 128], f32r, tag="fk")
            nc.vector.tensor_scalar(fk[:], fts[m][:, :, 0, :].bitcast(f32),
                                    bvec[:, m:m + 1], None, ALU.mult)
            fks[m] = fk

        def emit_feat(m):
            for bk_, t0, nt_, c in B2CHAINS:
                nc.tensor.matmul(psb[bk_][:, 0:256 * nt_], fks[m][:, c, :],
                                 fts[m][:, t0:t0 + nt_, :, :],
                                 start=(m == 0), stop=(m == FM - 1),
                                 skip_group_check=True)
            if m == 1:
                for bk_, t0, nt_, c in B2CHAINS:
                    nc.tensor.matmul(psb[bk_][:, 0:256 * nt_], ones2,
                                     negC2[:, 256 * t0:256 * (t0 + nt_)],
                                     start=False, stop=False,
                                     skip_group_check=True)

        def alloc_sb():
            return [ps.tile([128, 2, 256], f32, tag="psd0", bufs=1,
                            name="sb0"),
                    ps.tile([128, 2, 256], f32, tag="psd1", bufs=1,
                            name="sb1")]

        def emit_b01_scores(i, sbs, clo, chi):
            for c in range(clo, chi):
                for br in range(2):
                    lhs, rhs = (xke, xqe) if br == 0 else (kt, qt)
                    nc.tensor.matmul(sbs[br][:, c % 2, :],
                                     lhs[:, ts(c, 128)],
                                     rhs[0:65, ts(i, 256)],
                                     start=True, stop=True,
                                     skip_group_check=True)

        for m in range(FM):
            emit_head(m)
            if m >= 1:
                emit_sin(m - 1)
            if m >= 2:
                emit_fk(m - 2)
            if m >= 3:
                emit_feat(m - 3)
        emit_sin(FM - 1)
        emit_fk(FM - 2)
        emit_feat(FM - 3)
        # epilogue, interleaved with tile 3's first br0/br1 score pair
        sbs3 = alloc_sb()
        emit_fk(FM - 1)
        emit_feat(FM - 2)
        emit_b01_scores(3, sbs3, 0, 2)
        emit_feat(FM - 1)

        # ---- phase B: exp, mask, PV, drain ----
        pts2 = [None] * 6
        for bk_, t0, nt_, c in B2CHAINS:
            p2 = ptsp.tile([128, 512], f32r, tag=f"pts2{bk_}", bufs=1,
                           name=f"pts2{bk_}")
            nc.scalar.activation(p2[:, 0:256 * nt_], psb[bk_][:, 0:256 * nt_],
                                 AF.Exp)
            pts2[bk_] = p2
        for i, (bk_, off) in B2DIAG.items():
            nc.vector.tensor_tensor(pts2[bk_][:, off:off + 256],
                                    pts2[bk_][:, off:off + 256],
                                    tri01, ALU.mult)

        def pts_of(i, c):
            bk_, off = B2REG[(i, c)]
            return pts2[bk_][:, off:off + 256]

        # br2 PVs (banks psb0/psb1 reused after their exps)
        pv2a = ps.tile([65, 512], f32, tag="psb0", bufs=1, name="pv2a")
        nc.tensor.matmul(pv2a[:, 0:256], xkx[:, 0, :], pts_of(0, 0),
                         start=True, stop=True, skip_group_check=True)
        for c in range(2):
            nc.tensor.matmul(pv2a[:, 256:512], xkx[:, c, :], pts_of(1, c),
                             start=(c == 0), stop=(c == 1),
                             skip_group_check=True)
        ot2a = osp.tile([65, 512], f32, tag="ot2a", bufs=1, name="ot2a")
        nc.vector.tensor_copy(ot2a[:], pv2a[:])
        nc.sync.dma_start(d_o2[0], ot2a[:])
        pv2b = ps.tile([65, 512], f32, tag="psb1", bufs=1, name="pv2b")
        for c in range(3):
            nc.tensor.matmul(pv2b[:, 0:256], xkx[:, c, :], pts_of(2, c),
                             start=(c == 0), stop=(c == 2),
                             skip_group_check=True)
        for c in range(4):
            nc.tensor.matmul(pv2b[:, 256:512], xkx[:, c, :], pts_of(3, c),
                             start=(c == 0), stop=(c == 3),
                             skip_group_check=True)
        ot2b = osp.tile([65, 512], f32, tag="ot2b", bufs=1, name="ot2b")
        nc.vector.tensor_copy(ot2b[:], pv2b[:])
        nc.sync.dma_start(d_o2[1], ot2b[:])

        # br0/br1 per tile (big tiles first): chunk-paired exps
        for i in (3, 2, 1, 0):
            n = i + 1
            pv = ps.tile([65, 512], f32, tag="psb2", bufs=1, name="pv")
            sbs = sbs3 if i == 3 else alloc_sb()
            p01s = [ptsp.tile([128, NKC, 256], f32r, tag=f"p01{br}",
                              bufs=2, name=f"p01{br}") for br in range(2)]
            if i != 3:
                emit_b01_scores(i, sbs, 0, min(n, 2))
            w = min(n, 2)
            for br in range(2):
                nc.scalar.activation(p01s[br][:, 0:w, :], sbs[br][:, 0:w, :],
                                     AF.Exp)
            if n > 2:
                emit_b01_scores(i, sbs, 2, n)
                for br in range(2):
                    nc.scalar.activation(p01s[br][:, 2:n, :],
                                         sbs[br][:, 0:n - 2, :], AF.Exp)
            for br in range(2):
                nc.vector.tensor_tensor(p01s[br][:, i, :], p01s[br][:, i, :],
                                        tri01, ALU.mult)
            for br in range(2):
                vsrc = xkx if br == 0 else vte
                for c in range(n):
                    nc.tensor.matmul(pv[:, ts(br, 256)], vsrc[:, c, :],
                                     p01s[br][:, c, :],
                                     start=(c == 0), stop=(c == n - 1),
                                     skip_group_check=True)
            ot = osp.tile([65, 512], f32, tag="ot", bufs=3, name="ot")
            nc.vector.tensor_copy(ot[:], pv[:])
            nc.sync.dma_start(d_o01[i], ot[:])

    nc.compile()
    return nc


def _get_prog():
    if "nc" not in _prog_cache:
        _prog_cache["nc"] = _build_program()
    return _prog_cache["nc"]


def _perm_idx(role):
    perm = list(range(8)) if role == 0 else [1, 0, 3, 2, 5, 4, 7, 6]
    return np.concatenate([np.arange(128 * g, 128 * (g + 1)) for g in perm])


def _host_inputs(x, Wq, Wk, bk, Wv, attn_scale):
    """Build the 8 per-core input maps."""
    x = np.ascontiguousarray(np.asarray(x, dtype=np.float32))
    sc = float(np.asarray(attn_scale).reshape(-1)[0]) / np.sqrt(D)
    Wq = np.asarray(Wq, np.float32)
    Wk = np.asarray(Wk, np.float32)
    Wv = np.asarray(Wv, np.float32)
    bkc = np.asarray(bk, np.float32).reshape(D)

    wb = np.zeros((64, 193), np.float32)
    wb[:, 0:64] = Wq.T * sc
    wb[:, 64:128] = Wk.T
    wb[:, 128:192] = Wv.T
    wb[:, 192] = bkc

    # mask[partition=k, col=q] = 1 iff key k <= query q (within block)
    kk = np.arange(128)[:, None]
    qq = np.arange(128)[None, :]
    tril128 = (kk <= qq).astype(np.float32)

    # C bounds must cover every key the device exponentiates unmasked:
    # tile i processes key blocks up to 2i+1 (role 1), so cover through the
    # end of the odd block of each query's block pair.
    blk_end = np.minimum(128 * (((np.arange(S) // 128) | 1) + 1) - 1, S - 1)

    pmi = np.zeros((128, 256), np.float32)
    pmi[:, 0:128] = np.eye(128, dtype=np.float32)
    pmi[:, 128:256] = -np.eye(128, dtype=np.float32)

    in_maps = []
    for b in range(B):
        xb = x[b]                          # [S, D]

        nx = np.linalg.norm(xb, axis=1)
        C0 = nx * np.maximum.accumulate(nx)[blk_end] + 0.1
        qm = xb @ Wq.T * sc
        km = xb @ Wk.T + bkc
        C1 = (np.linalg.norm(qm, axis=1)
              * np.maximum.accumulate(np.linalg.norm(km, axis=1))[blk_end]
              + 0.1)
        Mblk = np.maximum.accumulate(xb, axis=0)[blk_end]
        C2 = np.tanh(xb + Mblk).sum(axis=1) + 0.5

        for role in range(2):
            pidx = _perm_idx(role)
            xpt = np.ascontiguousarray(xb[pidx].T)   # [D, S] permuted
            gblocks = [2 * c + role for c in range(NKC)]
            xk_g = np.concatenate([xb[128 * g:128 * g + 128] for g in gblocks])

            blob = np.zeros((128, BLOBW), np.float32)
            blob[0:64, OFF_PHI] = 0.125
            blob[64:128, OFF_PHI] = 0.375
            for mi in range(FM):
                blob[0:64, OFF_BV + mi] = BCOEF[mi]
                blob[64:128, OFF_BV + mi] = -BCOEF[mi]
            blob[:, OFF_TRI:OFF_TRI + 128] = tril128
            blob[:, OFF_TRI + 128:OFF_TRI + 256] = 1.0 if role == 0 else 0.0

            xqe = np.zeros((65, S), np.float32)
            xqe[0:64] = xpt
            xqe[64] = -C0[pidx]
            cr = np.zeros((33, CRW), np.float32)
            cr[0, 0:S] = -C1[pidx]
            cr[32, 0:S] = -C2[pidx]
            cr[0, S:] = 1.0
            cr[32, S:] = 1.0

            xke = np.ones((65, KL), np.float32)
            xke[0:64] = xk_g.T

            xkx = np.ones((128, NKC, 65), np.float32)
            xkx[:, :, 0:64] = xk_g.reshape(NKC, 128, D).transpose(1, 0, 2)

            in_maps.append({"blob": blob, "pmi": pmi, "xq2": xpt,
                            "xqe": xqe, "xke": xke, "xkx": xkx,
                            "wb": wb, "cr": cr})
    return in_maps


def _merge(results, attn_w):
    """Merge the two key-role partials per batch (shared C offsets)."""
    w = np.asarray(attn_w, np.float64)
    w = w / w.sum()
    out = np.zeros((B, S, D), np.float64)
    for b in range(B):
        for br in range(3):
            o = np.zeros((S, 64), np.float64)
            l = np.zeros(S, np.float64)
            for role in range(2):
                r = results[2 * b + role]
                pidx = _perm_idx(role)
                op = np.zeros((S, 64), np.float64)
                lp = np.zeros(S, np.float64)
                for i in range(NT):
                    if br < 2:
                        seg = r["o01"][i][:, 256 * br:256 * br + 256]
                    else:
                        seg = r["o2"][i // 2][:, 256 * (i % 2):
                                              256 * (i % 2) + 256]
                    op[QT * i:QT * (i + 1)] = seg[0:64].T
                    lp[QT * i:QT * (i + 1)] = seg[64]
                o[pidx] += op
                l[pidx] += lp
            out[b] += w[br] * (o / l[:, None])
    return out.astype(np.float32)


def kernel(x, Wq, Wk, bk, Wv, attn_w, attn_scale):
    global last_results
    from concourse.bass_utils import run_bass_kernel_spmd

    nc = _get_prog()
    in_maps = _host_inputs(x, Wq, Wk, bk, Wv, attn_scale)
    trace = os.environ.get("BASS_TRACE_KERNEL", "0") == "1"
    res = run_bass_kernel_spmd(nc, in_maps, core_ids=list(range(8)),
                               trace=trace)
    last_results = res
    return _merge(res.results, attn_w)


if __name__ == "__main__":
    rng = np.random.default_rng(0)
    xs = rng.standard_normal((B, S, D), dtype=np.float32)
    out = kernel(xs,
                 rng.standard_normal((D, D), dtype=np.float32) / 8,
                 rng.standard_normal((D, D), dtype=np.float32) / 8,
                 rng.standard_normal((D,), dtype=np.float32) / 8,
                 rng.standard_normal((D, D), dtype=np.float32) / 8,
                 np.ones(3, np.float32), np.ones(1, np.float32))
    print(out.shape, out.dtype)


# revision 17
# speedup vs baseline: 3.8555x; 1.0395x over previous
"""Trainium2 Bass kernel for nn_MultiAttention (3-branch causal attention).

Reference math (B=4, S=1024, D=64), per batch b:
  br0: s = x @ x^T                      ; causal softmax ; o = P @ x
  br1: s = (x Wq^T)(x Wk^T + bk)^T * sc ; causal softmax ; o = P @ (x Wv^T)
  br2: s[q,k] = sum_d tanh(x[q,d]+x[k,d]); causal softmax ; o = P @ x
  out = w0*o0 + w1*o1 + w2*o2,  w = attn_w/sum(attn_w)

Sharding: 8 cores = 4 batches x 2 key-roles. Core (b, r) handles all 1024
queries of batch b against the interleaved 128-key blocks {2c+r : c<4}.
The host permutes the query column order per core so key blocks always sit
at even block positions; the SPMD program is role-independent and the host
merge unpermutes.

Design (all scores computed TRANSPOSED, s^T[k, q], q-tiles of 256):
- No on-device row max: softmax uses host-computed per-query upper bounds
  C[q] (Cauchy-Schwarz / prefix-max bounds over each query's full diagonal
  block pair), subtracted inside the score matmul itself via an extra
  contraction row (ones x -C) or an in-chain rank-1 accumulate. Both
  key-roles share C so the host merge is a plain sum:
  out = (o_a + o_b) / (l_a + l_b).
- l comes free from PV: V is extended with a ones column, so PV's output
  row 64 is the softmax denominator.
- Causality: only each q-tile's diagonal chunk needs masking; applied as a
  0/1 multiply on P^T (post-exp) - C bounds cover the whole diagonal block
  pair so unmasked entries cannot overflow.
- Branch-2 runs as pure matmul via a free-frequency sine fit:
  tanh(z) ~ sum_m b_m sin(om_m z) (max err 1.2e-3 on |z|<=9.6, M=8), and
  sin(a+b) = sin(a+pi/4)sin(b+pi/4) - sin(a+3pi/4)sin(b+3pi/4), so one
  feature tile per m serves both q and k sides (keys are a gathered subset
  of query columns; the +/-b_m key scaling is one per-partition-scalar op).
  Range reduction per m: one DVE tensor_scalar (x/P + phase), one magic-
  number round, and the subtract split between the PE (+I/-I accumulate,
  slab 0) and Pool (tensor_tensor, slab 1).
- PSUM accumulation is chain-based (one OPEN chain per bank): br2 scores
  use 6 chains in banks 0-5 (adjacent tiles sharing a chunk pair into one
  512-wide chain); banks 6-7 (psd0/1) rotate for the d slabs, projections,
  and phase-B score tiles.
- Phase A is software-pipelined in emission order (fk lags one m, feature
  matmuls lag two) so no engine queue head-waits on its own iteration.
- ACT table thrash avoided: all Sin ops complete before any Exp op.
"""

import os
import sys

import numpy as np

try:
    import concourse.bass  # noqa: F401  (ambient install, e.g. under axon)
except ImportError:  # fall back to the in-container checkout
    for _p in ("/opt/trn_rl_repo",):
        if _p not in sys.path and os.path.isdir(_p):
            sys.path.insert(0, _p)

B, S, D = 4, 1024, 64
QT = 256                       # q-tile width
NT = S // QT                   # 4 q-tiles
NKC = 4                        # local key chunks per core
KL = NKC * 128                 # 512 local keys per core
FM = 8                         # sine-series terms
MAGIC = 12582912.0             # 1.5 * 2**23: fp32 round-to-nearest trick
SSCALE = float(2.0 * np.pi * (1.0 - 5e-7))

# free-frequency LSQ fit of tanh on [0, 9.6] (max err 1.21e-3)
OMEGAS = [0.2734280786, 0.8243559956, 1.3856134054, 1.9598657311,
          2.5472323275, 3.1465182453, 3.7546312203, 4.3568228756]
BCOEF = [1.23654055, 0.3289342548, 0.1304462844, 0.0535883686,
         0.0217261607, 0.0086277304, 0.0033462421, 0.001215308]

# br2 chain groups: one open PSUM accumulation chain per bank.
# (bank, first_tile, n_tiles, chunk): out width = 256*n_tiles
B2CHAINS = [(0, 0, 2, 0),   # tiles 0-1, chunk 0
            (1, 2, 2, 0),   # tiles 2-3, chunk 0
            (2, 2, 2, 1),   # tiles 2-3, chunk 1
            (3, 2, 2, 2),   # tiles 2-3, chunk 2
            (4, 1, 1, 1),   # tile 1, chunk 1 (diag)
            (5, 3, 1, 3)]   # tile 3, chunk 3 (diag)
B2REG = {}
for _bk, _t0, _nt, _c in B2CHAINS:
    for _j in range(_nt):
        B2REG[(_t0 + _j, _c)] = (_bk, 256 * _j)
B2DIAG = {i: B2REG[(i, i)] for i in range(NT)}

# blobr (f32r): xq2 | +I | -I ; blob (f32): phi | bvec | tri01
OFF_PI = S
OFF_NI = S + 128
BLOBRW = S + 256
OFF_PHI = 0
OFF_BV = 1
OFF_TRI = 1 + FM
BLOBW = 1 + FM + 256
CRW = S + 128 + KL             # crows: -C1/-C2 | ones128 | ones512

_prog_cache = {}
last_results = None  # BassKernelResults of the most recent run (for test.py)


def _build_program():
    import concourse.bacc as bacc
    import concourse.bass as bass
    import concourse.mybir as mybir
    import concourse.tile as tile
    from contextlib import ExitStack

    f32 = mybir.dt.float32
    f32r = mybir.dt.float32r
    AF = mybir.ActivationFunctionType
    ALU = mybir.AluOpType
    ts = bass.ts

    nc = bacc.Bacc("TRN2", target_bir_lowering=False, debug=False,
                   num_devices=8)

    d_blob = nc.dram_tensor("blob", [128, BLOBW], f32,
                            kind="ExternalInput").ap()
    d_xq2 = nc.dram_tensor("xq2", [64, S], f32r, kind="ExternalInput").ap()
    d_wx = nc.dram_tensor("wx", [65, KL + 193], f32r,
                          kind="ExternalInput").ap()
    d_xp = nc.dram_tensor("xp", [128, NKC * 65 + 256], f32r,
                          kind="ExternalInput").ap()
    d_xc = nc.dram_tensor("xc", [65, S + CRW], f32r,
                          kind="ExternalInput").ap()

    d_o01 = nc.dram_tensor("o01", [NT, 65, 512], f32,
                           kind="ExternalOutput").ap()
    d_o2 = nc.dram_tensor("o2", [2, 65, 512], f32,
                          kind="ExternalOutput").ap()

    with tile.TileContext(nc) as tc, ExitStack() as ctx:
        consts = ctx.enter_context(tc.tile_pool(name="consts", bufs=1))
        a2p = ctx.enter_context(tc.tile_pool(name="a2p", bufs=2))
        rmp = ctx.enter_context(tc.tile_pool(name="rmp", bufs=2))
        d1p = ctx.enter_context(tc.tile_pool(name="d1p", bufs=2))
        ftp = ctx.enter_context(tc.tile_pool(name="ftp", bufs=5))
        fkp = ctx.enter_context(tc.tile_pool(name="fkp", bufs=5))
        ptsp = ctx.enter_context(tc.tile_pool(name="ptsp", bufs=1))
        osp = ctx.enter_context(tc.tile_pool(name="osp", bufs=1))
        ps = ctx.enter_context(tc.tile_pool(name="ps", bufs=1, space="PSUM"))

        # DMAs in priority order (xq2 + blob gate the m-loop)
        xq2 = consts.tile([128, S], f32r, tag="xq2")
        nc.sync.dma_start(xq2[0:64, :], d_xq2)
        nc.sync.dma_start(xq2[64:128, :], d_xq2)
        blob = consts.tile([128, BLOBW], f32, tag="blob")
        nc.sync.dma_start(blob[:], d_blob)
        wx = consts.tile([65, KL + 193], f32r, tag="wx")
        nc.sync.dma_start(wx[:], d_wx)
        xp = consts.tile([128, NKC * 65 + 256], f32r, tag="xp")
        nc.sync.dma_start(xp[:], d_xp)
        xc = consts.tile([65, S + CRW], f32r, tag="xc")
        nc.sync.dma_start(xc[:], d_xc)

        xke = wx[:, 0:KL]
        wb = wx[0:64, KL:KL + 193]
        xkx = xp[:, 0:NKC * 65].rearrange("p (a b) -> p a b", a=NKC)
        xqe = xc[:, 0:S]

        qt = consts.tile([65, S], f32r, tag="qt")
        nc.gpsimd.tensor_copy(qt[64:65, :], xc[0:1, S:2 * S])     # -C1
        kt = consts.tile([65, KL], f32r, tag="kt")
        nc.gpsimd.tensor_copy(kt[64:65, :], xc[0:1, 2 * S + 128:])  # ones
        vte = consts.tile([128, NKC, 65], f32r, tag="vte")

        phi = blob[:, OFF_PHI:OFF_PHI + 1]
        bvec = blob[:, OFF_BV:OFF_BV + FM]
        tri01 = blob[:, OFF_TRI:OFF_TRI + 256].bitcast(f32r)
        posI = xp[:, NKC * 65:NKC * 65 + 128]
        negI = xp[:, NKC * 65 + 128:NKC * 65 + 256]
        negC2 = xc[32:33, S:2 * S]         # -C2 row (base partition 32)
        ones2 = xc[32:33, 2 * S:2 * S + 128]  # ones, base 32

        # br2 score banks 0-5; psd0/1 rotate for d slabs / proj / phase B
        psb = [ps.tile([128, 512], f32, tag=f"psb{i}", bufs=1, name=f"psb{i}")
               for i in range(6)]

        def psd(k, shape=[128, 2, 2, 128]):
            return ps.tile(shape, f32, tag=f"psd{k % 2}", bufs=1,
                           name=f"psd{k % 2}")

        # ---- projections (fills early PE idle; PSUM from psb banks,
        # which stay free until their br2 chains open at feat(0)) ----
        # qt = Wq' x^T (scaled), kt = Wk' x^T + bk, vte = x Wv^T | 1
        qps = []
        for h in range(2):
            qp = ps.tile([64, 512], f32, tag=f"psb{h}", bufs=1, name="qp")
            nc.tensor.matmul(qp[:], wb[:, 0:64], xq2[0:64, ts(h, 512)],
                             start=True, stop=True)
            qps.append(qp)
        kp = ps.tile([64, KL], f32, tag="psb2", bufs=1, name="kp")
        nc.tensor.matmul(kp[:], wb[:, 64:128], xke[0:64, :],
                         start=True, stop=True)
        vp = ps.tile([128, 256], f32, tag="psb3", bufs=1, name="vp")
        for c in range(NKC):
            nc.tensor.matmul(vp[:, ts(c, 64)], xke[0:64, ts(c, 128)],
                             wb[:, 128:192], start=True, stop=True)
        for h in range(2):
            nc.scalar.activation(qt[0:64, ts(h, 512)], qps[h][:],
                                 AF.Identity)
        nc.scalar.activation(kt[0:64, :], kp[:], AF.Identity,
                             bias=wb[:, 192:193].bitcast(f32))
        for c in range(NKC):
            nc.vector.tensor_copy(vte[:, c, 0:64], vp[:, ts(c, 64)])
        nc.vector.tensor_copy(vte[:, :, 64:65], xkx[:, :, 64:65])

        # ---- feature m-loop (phase A), software-pipelined emission ----
        # a = x/P + phase ; r = round(a) [magic] ; d = a - r (slab0 on PE
        # via +I/-I, slab1 on Pool) ; f = sin(2*pi*d) ; fk = (+/-b_m)*f[key]
        a2s, rms, d0s, d1s, fts, fks = {}, {}, {}, {}, {}, {}

        def emit_head(m):
            pm = 2.0 * np.pi / OMEGAS[m]
            a2 = a2p.tile([128, S], f32r, tag="a2")
            nc.vector.tensor_scalar(a2[:], xq2[:].bitcast(f32),
                                    float(1.0 / pm), phi[:, 0:1],
                                    ALU.mult, ALU.add)
            rm = rmp.tile([128, S], f32r, tag="rm")
            nc.vector.tensor_scalar(rm[:], a2[:].bitcast(f32),
                                    MAGIC, MAGIC, ALU.add, ALU.subtract)
            a2s[m], rms[m] = a2, rm
            dt0 = psd(m)
            nc.tensor.matmul(dt0[:], posI, a2[:, 0:512],
                             start=True, stop=False, skip_group_check=True)
            nc.tensor.matmul(dt0[:], negI, rm[:, 0:512],
                             start=False, stop=True, skip_group_check=True)
            d0s[m] = dt0
            dt1 = d1p.tile([128, 2, 2, 128], f32, tag="d1")
            nc.gpsimd.tensor_tensor(dt1[:].rearrange("p a b c -> p (a b c)"),
                                    a2[:, 512:1024].bitcast(f32),
                                    rm[:, 512:1024].bitcast(f32),
                                    ALU.subtract)
            d1s[m] = dt1

        def emit_sin(m):
            ft = ftp.tile([128, NT, 2, 128], f32r, tag="ft")
            nc.scalar.activation(ft[:, 0:2, :, :], d0s[m][:], AF.Sin,
                                 scale=SSCALE)
            nc.scalar.activation(ft[:, 2:4, :, :], d1s[m][:], AF.Sin,
                                 scale=SSCALE)
            fts[m] = ft

        def emit_fk(m):
            fk = fkp.tile([128, NKC, 128], f32r, tag="fk")
            nc.vector.tensor_scalar(fk[:], fts[m][:, :, 0, :].bitcast(f32),
                                    bvec[:, m:m + 1], None, ALU.mult)
            fks[m] = fk

        def emit_feat(m):
            for bk_, t0, nt_, c in B2CHAINS:
                nc.tensor.matmul(psb[bk_][:, 0:256 * nt_], fks[m][:, c, :],
                                 fts[m][:, t0:t0 + nt_, :, :],
                                 start=(m == 0), stop=(m == FM - 1),
                                 skip_group_check=True)
            if m == 1:
                for bk_, t0, nt_, c in B2CHAINS:
                    nc.tensor.matmul(psb[bk_][:, 0:256 * nt_], ones2,
                                     negC2[:, 256 * t0:256 * (t0 + nt_)],
                                     start=False, stop=False,
                                     skip_group_check=True)

        def alloc_sb(alt):
            tags = ("psd0", "psd1") if not alt else ("psb3", "psb5")
            return [ps.tile([128, 2, 256], f32, tag=tags[0], bufs=1,
                            name="sb0"),
                    ps.tile([128, 2, 256], f32, tag=tags[1], bufs=1,
                            name="sb1")]

        def emit_b01_scores(i, sbs, clo, chi):
            for c in range(clo, chi):
                for br in range(2):
                    lhs, rhs = (xke, xqe) if br == 0 else (kt, qt)
                    nc.tensor.matmul(sbs[br][:, c % 2, :],
                                     lhs[:, ts(c, 128)],
                                     rhs[0:65, ts(i, 256)],
                                     start=True, stop=True,
                                     skip_group_check=True)

        for m in range(FM):
            emit_head(m)
            if m >= 1:
                emit_sin(m - 1)
            if m >= 2:
                emit_fk(m - 2)
            if m >= 3:
                emit_feat(m - 3)
        emit_sin(FM - 1)
        emit_fk(FM - 2)
        emit_feat(FM - 3)
        # epilogue, interleaved with tile 3's first br0/br1 score pair
        sbs3 = alloc_sb(False)
        emit_fk(FM - 1)
        emit_feat(FM - 2)
        emit_b01_scores(3, sbs3, 0, 2)
        emit_feat(FM - 1)

        # ---- phase B: exp, mask, PV, drain ----
        pts2 = [None] * 6
        for bk_, t0, nt_, c in B2CHAINS:
            p2 = ptsp.tile([128, 512], f32r, tag=f"pts2{bk_}", bufs=1,
                           name=f"pts2{bk_}")
            nc.scalar.activation(p2[:, 0:256 * nt_], psb[bk_][:, 0:256 * nt_],
                                 AF.Exp)
            pts2[bk_] = p2
        for i, (bk_, off) in B2DIAG.items():
            nc.vector.tensor_tensor(pts2[bk_][:, off:off + 256],
                                    pts2[bk_][:, off:off + 256],
                                    tri01, ALU.mult)

        def pts_of(i, c):
            bk_, off = B2REG[(i, c)]
            return pts2[bk_][:, off:off + 256]

        # br2 PVs (banks psb0/psb1 reused after their exps)
        pv2a = ps.tile([65, 512], f32, tag="psb0", bufs=1, name="pv2a")
        nc.tensor.matmul(pv2a[:, 0:256], xkx[:, 0, :], pts_of(0, 0),
                         start=True, stop=True, skip_group_check=True)
        for c in range(2):
            nc.tensor.matmul(pv2a[:, 256:512], xkx[:, c, :], pts_of(1, c),
                             start=(c == 0), stop=(c == 1),
                             skip_group_check=True)
        ot2a = osp.tile([65, 512], f32, tag="ot2a", bufs=1, name="ot2a")
        nc.vector.tensor_copy(ot2a[:], pv2a[:])
        nc.sync.dma_start(d_o2[0], ot2a[:])
        pv2b = ps.tile([65, 512], f32, tag="psb1", bufs=1, name="pv2b")
        for c in range(3):
            nc.tensor.matmul(pv2b[:, 0:256], xkx[:, c, :], pts_of(2, c),
                             start=(c == 0), stop=(c == 2),
                             skip_group_check=True)
        for c in range(4):
            nc.tensor.matmul(pv2b[:, 256:512], xkx[:, c, :], pts_of(3, c),
                             start=(c == 0), stop=(c == 3),
                             skip_group_check=True)
        ot2b = osp.tile([65, 512], f32, tag="ot2b", bufs=1, name="ot2b")
        nc.vector.tensor_copy(ot2b[:], pv2b[:])
        nc.sync.dma_start(d_o2[1], ot2b[:])

        # br0/br1 per tile (big tiles first): chunk-paired exps
        for i in (3, 2, 1, 0):
            n = i + 1
            pv = ps.tile([65, 512], f32, tag="psb2" if i % 2 else "psb4",
                         bufs=1, name="pv")
            sbs = sbs3 if i == 3 else alloc_sb(i % 2 == 0)
            p01s = [ptsp.tile([128, NKC, 256], f32r, tag=f"p01{br}",
                              bufs=2, name=f"p01{br}") for br in range(2)]
            if i != 3:
                emit_b01_scores(i, sbs, 0, min(n, 2))
            w = min(n, 2)
            for br in range(2):
                nc.scalar.activation(p01s[br][:, 0:w, :], sbs[br][:, 0:w, :],
                                     AF.Exp)
            if n > 2:
                emit_b01_scores(i, sbs, 2, n)
                for br in range(2):
                    nc.scalar.activation(p01s[br][:, 2:n, :],
                                         sbs[br][:, 0:n - 2, :], AF.Exp)
            for br in range(2):
                nc.vector.tensor_tensor(p01s[br][:, i, :], p01s[br][:, i, :],
                                        tri01, ALU.mult)
            for br in range(2):
                vsrc = xkx if br == 0 else vte
                for c in range(n):
                    nc.tensor.matmul(pv[:, ts(br, 256)], vsrc[:, c, :],
                                     p01s[br][:, c, :],
                                     start=(c == 0), stop=(c == n - 1),
                                     skip_group_check=True)
            ot = osp.tile([65, 512], f32, tag="ot", bufs=3, name="ot")
            if i == 0:
                nc.scalar.activation(ot[:], pv[:], AF.Identity)
            else:
                nc.vector.tensor_copy(ot[:], pv[:])
            nc.sync.dma_start(d_o01[i], ot[:])

    nc.compile()
    return nc


def _get_prog():
    if "nc" not in _prog_cache:
        _prog_cache["nc"] = _build_program()
    return _prog_cache["nc"]


def _perm_idx(role):
    perm = list(range(8)) if role == 0 else [1, 0, 3, 2, 5, 4, 7, 6]
    return np.concatenate([np.arange(128 * g, 128 * (g + 1)) for g in perm])


def _host_inputs(x, Wq, Wk, bk, Wv, attn_scale):
    """Build the 8 per-core input maps."""
    x = np.ascontiguousarray(np.asarray(x, dtype=np.float32))
    sc = float(np.asarray(attn_scale).reshape(-1)[0]) / np.sqrt(D)
    Wq = np.asarray(Wq, np.float32)
    Wk = np.asarray(Wk, np.float32)
    Wv = np.asarray(Wv, np.float32)
    bkc = np.asarray(bk, np.float32).reshape(D)

    wb = np.zeros((64, 193), np.float32)
    wb[:, 0:64] = Wq.T * sc
    wb[:, 64:128] = Wk.T
    wb[:, 128:192] = Wv.T
    wb[:, 192] = bkc

    # mask[partition=k, col=q] = 1 iff key k <= query q (within block)
    kk = np.arange(128)[:, None]
    qq = np.arange(128)[None, :]
    tril128 = (kk <= qq).astype(np.float32)

    # C bounds must cover every key the device exponentiates unmasked:
    # tile i processes key blocks up to 2i+1 (role 1), so cover through the
    # end of the odd block of each query's block pair.
    blk_end = np.minimum(128 * (((np.arange(S) // 128) | 1) + 1) - 1, S - 1)

    pmi = np.zeros((128, 256), np.float32)
    pmi[:, 0:128] = np.eye(128, dtype=np.float32)
    pmi[:, 128:256] = -np.eye(128, dtype=np.float32)
    wxw = np.zeros((65, KL + 193), np.float32)
    wxw[0:64, KL:] = wb

    in_maps = []
    for b in range(B):
        xb = x[b]                          # [S, D]

        nx = np.linalg.norm(xb, axis=1)
        C0 = nx * np.maximum.accumulate(nx)[blk_end] + 0.1
        qm = xb @ Wq.T * sc
        km = xb @ Wk.T + bkc
        C1 = (np.linalg.norm(qm, axis=1)
              * np.maximum.accumulate(np.linalg.norm(km, axis=1))[blk_end]
              + 0.1)
        Mblk = np.maximum.accumulate(xb, axis=0)[blk_end]
        C2 = np.tanh(xb + Mblk).sum(axis=1) + 0.5

        for role in range(2):
            pidx = _perm_idx(role)
            xpt = np.ascontiguousarray(xb[pidx].T)   # [D, S] permuted
            gblocks = [2 * c + role for c in range(NKC)]
            xk_g = np.concatenate([xb[128 * g:128 * g + 128] for g in gblocks])

            blob = np.zeros((128, BLOBW), np.float32)
            blob[0:64, OFF_PHI] = 0.125
            blob[64:128, OFF_PHI] = 0.375
            for mi in range(FM):
                blob[0:64, OFF_BV + mi] = BCOEF[mi]
                blob[64:128, OFF_BV + mi] = -BCOEF[mi]
            blob[:, OFF_TRI:OFF_TRI + 128] = tril128
            blob[:, OFF_TRI + 128:OFF_TRI + 256] = 1.0 if role == 0 else 0.0

            xc = np.zeros((65, S + CRW), np.float32)
            xc[0:64, 0:S] = xpt
            xc[64, 0:S] = -C0[pidx]
            xc[0, S:2 * S] = -C1[pidx]
            xc[32, S:2 * S] = -C2[pidx]
            xc[0, 2 * S:] = 1.0
            xc[32, 2 * S:] = 1.0

            wx = wxw.copy()
            wx[:, 0:KL] = 1.0
            wx[0:64, 0:KL] = xk_g.T

            xp = np.zeros((128, NKC * 65 + 256), np.float32)
            xkx = np.ones((128, NKC, 65), np.float32)
            xkx[:, :, 0:64] = xk_g.reshape(NKC, 128, D).transpose(1, 0, 2)
            xp[:, 0:NKC * 65] = xkx.reshape(128, NKC * 65)
            xp[:, NKC * 65:] = pmi

            in_maps.append({"blob": blob, "xq2": xpt, "wx": wx,
                            "xp": xp, "xc": xc})
    return in_maps


def _merge(results, attn_w):
    """Merge the two key-role partials per batch (shared C offsets)."""
    w = np.asarray(attn_w, np.float64)
    w = w / w.sum()
    out = np.zeros((B, S, D), np.float64)
    for b in range(B):
        for br in range(3):
            o = np.zeros((S, 64), np.float64)
            l = np.zeros(S, np.float64)
            for role in range(2):
                r = results[2 * b + role]
                pidx = _perm_idx(role)
                op = np.zeros((S, 64), np.float64)
                lp = np.zeros(S, np.float64)
                for i in range(NT):
                    if br < 2:
                        seg = r["o01"][i][:, 256 * br:256 * br + 256]
                    else:
                        seg = r["o2"][i // 2][:, 256 * (i % 2):
                                              256 * (i % 2) + 256]
                    op[QT * i:QT * (i + 1)] = seg[0:64].T
                    lp[QT * i:QT * (i + 1)] = seg[64]
                o[pidx] += op
                l[pidx] += lp
            out[b] += w[br] * (o / l[:, None])
    return out.astype(np.float32)


def kernel(x, Wq, Wk, bk, Wv, attn_w, attn_scale):
    global last_results
    from concourse.bass_utils import run_bass_kernel_spmd

    nc = _get_prog()
    in_maps = _host_inputs(x, Wq, Wk, bk, Wv, attn_scale)
    trace = os.environ.get("BASS_TRACE_KERNEL", "0") == "1"
    res = run_bass_kernel_spmd(nc, in_maps, core_ids=list(range(8)),
                               trace=trace)
    last_results = res
    return _merge(res.results, attn_w)


if __name__ == "__main__":
    rng = np.random.default_rng(0)
    xs = rng.standard_normal((B, S, D), dtype=np.float32)
    out = kernel(xs,
                 rng.standard_normal((D, D), dtype=np.float32) / 8,
                 rng.standard_normal((D, D), dtype=np.float32) / 8,
                 rng.standard_normal((D,), dtype=np.float32) / 8,
                 rng.standard_normal((D, D), dtype=np.float32) / 8,
                 np.ones(3, np.float32), np.ones(1, np.float32))
    print(out.shape, out.dtype)
